# revision 7
# baseline (speedup 1.0000x reference)
"""Distributed multi-head GAT (encoder + 2 GAT layers) on 8 TRN2 NeuronCores.

Strategy (graph/data parallel, per the dst-ownership sharding):
  * Nodes are permuted and dealt into 8*NT bins of 128 nodes so that every
    bin (= one PSUM dst-tile) has a near-equal number of incoming edges and
    every core has a near-equal total.  Edges live with the core that owns
    their dst node.
  * Each core encodes its own node shard (obs -> z1), builds a per-node
    table row [feat(128) | el(8)], and the 8 shards are AllGathered into a
    full table so any core can gather src features ("halo" = full table).
  * Edge pass: for each 128-edge tile, indirect-DMA-gather the src table
    rows, gather er[dst], compute ex = exp(leaky_relu(el+er)) (exactly
    max(exp(x), exp(0.2x))), scale messages, and reduce into the dst-tile
    PSUM accumulator with a one-hot matmul (Bt[e,d] = [dstpos[e] == d]).
    The softmax max-subtraction is skipped: inputs are O(0.1) so exp is
    safe, and the result is identical up to the 1e-9 epsilon scaling.
  * Epilogue per dst-tile: out = relu(acc_feat) * 1/(acc_z + 1e-9) per
    head, which also builds the next layer's table row + er entries.
"""

import os
import sys
import time

import numpy as np

for _p in ("/opt/trn_rl_repo", "/root/.axon_site/_ro/trn_rl_repo"):
    if os.path.isdir(_p) and _p not in sys.path:
        sys.path.insert(0, _p)

P = 128
NCORES = 8
OBS_D = 256
HID = 512
H = 128          # h_dim
NH = 8           # heads
HD = 16          # head dim
TAB = H + NH     # table row: feat | el
PAD_SENTINEL = 200.0

LAST_INFO = {}


def _ensure_ntff_hook():
    """Register the axon NTFF profile hook if the image's antenv lacks it."""
    try:
        import types

        import antenv
        try:
            from antenv import axon_hooks  # noqa: F401
            return
        except ImportError:
            pass
        m = types.ModuleType("antenv.axon_hooks")
        _h = [None]
        m.set_axon_ntff_profile_hook = lambda hook: _h.__setitem__(0, hook)
        m.get_axon_ntff_profile_hook = lambda: _h[0]
        sys.modules["antenv.axon_hooks"] = m
        antenv.axon_hooks = m
        from trn_agent_boot.trn_boot import _ntff_profile_via_ctypes
        m.set_axon_ntff_profile_hook(
            _ntff_profile_via_ctypes("/opt/axon/libaxon_pjrt.so"))
    except Exception as e:  # profiling is best-effort
        print(f"ntff hook setup failed: {e}")


# ----------------------------------------------------------------------------
# Host-side preprocessing: balance nodes into bins, build edge slabs
# ----------------------------------------------------------------------------

def _host_prepare(src, dst, n_tiles_per_core):
    """Returns permutation + per-core int slabs. Pure index manipulation."""
    NT = n_tiles_per_core
    NPC = NT * P
    NTOT = NCORES * NPC
    NBINS = NCORES * NT
    E = src.shape[0]

    deg = np.bincount(dst, minlength=NTOT).astype(np.int64)
    order = np.argsort(-deg, kind="stable")
    # serpentine deal into NBINS bins x 128 rounds -> balanced bin edge counts
    arr = order.reshape(P, NBINS).copy()
    arr[1::2] = arr[1::2, ::-1]
    # bin b holds nodes arr[:, b]; permuted-global id g = b*128 + r
    perm = arr.T.reshape(-1)                       # g -> orig node
    pos = np.empty(NTOT, np.int64)
    pos[perm] = np.arange(NTOT)                    # orig node -> g

    binload = deg[arr].sum(axis=0)                 # [NBINS]
    T = max(1, int(np.ceil(binload.max() / P)))

    dstg = pos[dst]
    binid = dstg // P
    eorder = np.argsort(binid, kind="stable")
    counts = np.bincount(binid, minlength=NBINS)
    starts = np.concatenate([[0], np.cumsum(counts)[:-1]])
    rank = np.arange(E) - starts[binid[eorder]]
    slot = binid[eorder] * (T * P) + rank

    ES = NBINS * T * P
    srcix = np.zeros(ES, np.int32)
    dlocal = np.zeros(ES, np.int32)
    dposf = np.full(ES, PAD_SENTINEL, np.float32)
    srcix[slot] = pos[src[eorder]].astype(np.int32)
    dlocal[slot] = (dstg[eorder] % NPC).astype(np.int32)
    dposf[slot] = (dstg[eorder] % P).astype(np.float32)

    def slabs(a):
        # [NBINS*T*128] -> per core [128, NT*T] (col = edge tile, row = lane)
        a = a.reshape(NCORES, NT * T, P).transpose(0, 2, 1)
        return [np.ascontiguousarray(a[c]) for c in range(NCORES)]

    return dict(
        perm=perm, pos=pos, T=T, NPC=NPC, NTOT=NTOT,
        srcix=slabs(srcix), dlocal=slabs(dlocal), dposf=slabs(dposf),
        binload=binload,
    )


# ----------------------------------------------------------------------------
# Device program
# ----------------------------------------------------------------------------

def _build_program(NT, T):
    import concourse.bacc as bacc
    import concourse.bass as bass
    import concourse.mybir as mybir
    import concourse.tile as tile

    dt = mybir.dt
    F = dt.float32r      # 4-byte float, fast PE mode
    F32 = dt.float32
    I32 = dt.int32
    AF = mybir.ActivationFunctionType
    OP = mybir.AluOpType

    NPC = NT * P
    NTOT = NCORES * NPC
    NETILES = NT * T

    nc = bacc.Bacc("TRN2", target_bir_lowering=False, debug=False,
                   num_devices=NCORES)

    # ---- parameters (per-core values supplied via in_maps) ----
    obst_p = nc.dram_tensor("obst", [OBS_D, NPC], F, kind="ExternalInput")
    W1_p = nc.dram_tensor("w1", [OBS_D, HID], F, kind="ExternalInput")
    b1_p = nc.dram_tensor("b1", [HID, 1], F32, kind="ExternalInput")
    W2_p = nc.dram_tensor("w2", [HID, H], F, kind="ExternalInput")
    b2_p = nc.dram_tensor("b2", [H, 1], F32, kind="ExternalInput")
    Wg_p = [nc.dram_tensor(f"wg{i}", [H, H], F, kind="ExternalInput")
            for i in (1, 2)]
    Wgal_p = [nc.dram_tensor(f"wgal{i}", [H, NH], F, kind="ExternalInput")
              for i in (1, 2)]
    Wgar_p = [nc.dram_tensor(f"wgar{i}", [H, NH], F, kind="ExternalInput")
              for i in (1, 2)]
    iota_p = nc.dram_tensor("iota", [P, P], F, kind="ExternalInput")
    iotac_p = nc.dram_tensor("iotac", [P, 1], F, kind="ExternalInput")
    srcs_p = nc.dram_tensor("srcslab", [P, NETILES], I32, kind="ExternalInput")
    dloc_p = nc.dram_tensor("dlocslab", [P, NETILES], I32, kind="ExternalInput")
    dpos_p = nc.dram_tensor("dposslab", [P, NETILES], F, kind="ExternalInput")
    out_p = nc.dram_tensor("out", [NPC, 3 * H], F, kind="ExternalOutput")

    # ---- internal DRAM ----
    tab_loc = [nc.dram_tensor(f"tab{i}_loc", [NPC, TAB], F) for i in (1, 2)]
    tab_full = [nc.dram_tensor(f"tab{i}_full", [NTOT, TAB], F,
                               addr_space="Shared") for i in (1, 2)]
    er_tab = [nc.dram_tensor(f"er{i}_tab", [NPC, NH], F) for i in (1, 2)]

    groups = [list(range(NCORES))]

    with tile.TileContext(nc) as tc:
        with (
            tc.tile_pool(name="const", bufs=1) as constp,
            tc.tile_pool(name="obst", bufs=3) as obstp,
            tc.tile_pool(name="enc", bufs=3) as encp,
            tc.tile_pool(name="rows", bufs=3) as rowsp,
            tc.tile_pool(name="gath", bufs=8) as gathp,
            tc.tile_pool(name="small", bufs=8) as smallp,
            tc.tile_pool(name="rhs", bufs=6) as rhsp,
            tc.tile_pool(name="bt", bufs=6) as btp,
            tc.tile_pool(name="pe", bufs=3, space="PSUM") as pep,
            tc.tile_pool(name="pacc", bufs=2, space="PSUM") as paccp,
            tc.tile_pool(name="ptr", bufs=1, space="PSUM") as ptrp,
            tc.tile_pool(name="prod", bufs=2, space="PSUM") as prodp,
        ):
            # ---------------- prologue: constants & weights ----------------
            iota_sb = constp.tile([P, P], F, tag="iota")
            nc.sync.dma_start(iota_sb[:], iota_p[:, :])
            iotac_sb = constp.tile([P, 1], F, tag="iotac")
            nc.sync.dma_start(iotac_sb[:], iotac_p[:, :])
            ident = constp.tile([P, P], F, tag="ident")
            nc.vector.tensor_tensor(out=ident[:],
                                    in0=iotac_sb[:, 0:1].to_broadcast([P, P]),
                                    in1=iota_sb[:], op=OP.is_equal)

            W1_sb = []
            for k in range(2):
                t = constp.tile([P, HID], F, tag=f"w1_{k}")
                nc.sync.dma_start(t[:], W1_p[k * P:(k + 1) * P, :])
                W1_sb.append(t)
            W2_sb = []
            for m in range(4):
                t = constp.tile([P, H], F, tag=f"w2_{m}")
                nc.sync.dma_start(t[:], W2_p[m * P:(m + 1) * P, :])
                W2_sb.append(t)
            b1_sb = []
            for m in range(4):
                t = constp.tile([P, 1], F32, tag=f"b1_{m}")
                nc.sync.dma_start(t[:], b1_p[m * P:(m + 1) * P, :])
                b1_sb.append(t)
            b2_sb = constp.tile([P, 1], F32, tag="b2")
            nc.sync.dma_start(b2_sb[:], b2_p[:, :])
            Wg_sb, Wgal_sb, Wgar_sb = [], [], []
            for i in range(2):
                t = constp.tile([P, H], F, tag=f"wg_{i}")
                nc.sync.dma_start(t[:], Wg_p[i][:, :])
                Wg_sb.append(t)
                t = constp.tile([P, NH], F, tag=f"wgal_{i}")
                nc.sync.dma_start(t[:], Wgal_p[i][:, :])
                Wgal_sb.append(t)
                t = constp.tile([P, NH], F, tag=f"wgar_{i}")
                nc.sync.dma_start(t[:], Wgar_p[i][:, :])
                Wgar_sb.append(t)
            srcs_sb = constp.tile([P, NETILES], I32, tag="srcs")
            nc.sync.dma_start(srcs_sb[:], srcs_p[:, :])
            dloc_sb = constp.tile([P, NETILES], I32, tag="dloc")
            nc.sync.dma_start(dloc_sb[:], dloc_p[:, :])
            dpos_sb = constp.tile([P, NETILES], F, tag="dpos")
            nc.sync.dma_start(dpos_sb[:], dpos_p[:, :])

            # helper: from z.T chunk [128h, 128n] at node row base, build
            # table rows / er rows for layer li and write z rows to out col.
            def node_tile_products(zT_chunk, row0, li, out_col, z_rows_tile):
                # z rows -> out
                ptr = ptrp.tile([P, P], F, tag="ptr")
                nc.tensor.transpose(ptr[:], zT_chunk, ident[:])
                zr = rowsp.tile([P, P], F, tag="zrows")
                nc.vector.tensor_copy(zr[:], ptr[:])
                nc.sync.dma_start(
                    out_p[row0:row0 + P, out_col:out_col + H], zr[:])
                if li is None:
                    return
                # feat rows / el rows / er rows via lhsT = zT chunk, one bank
                pr = prodp.tile([P, H + 2 * NH], F32, tag="pr")
                nc.tensor.matmul(pr[:, 0:H], lhsT=zT_chunk, rhs=Wg_sb[li][:],
                                 start=True, stop=True)
                nc.tensor.matmul(pr[:, H:H + NH], lhsT=zT_chunk,
                                 rhs=Wgal_sb[li][:], start=True, stop=True)
                nc.tensor.matmul(pr[:, H + NH:H + 2 * NH], lhsT=zT_chunk,
                                 rhs=Wgar_sb[li][:], start=True, stop=True)
                tabt = rowsp.tile([P, TAB], F, tag="tabt")
                nc.vector.tensor_copy(tabt[:], pr[:, 0:TAB])
                nc.sync.dma_start(tab_loc[li][row0:row0 + P, :], tabt[:])
                ert = rowsp.tile([P, NH], F, tag="ert")
                nc.vector.tensor_copy(ert[:], pr[:, H + NH:H + 2 * NH])
                nc.sync.dma_start(er_tab[li][row0:row0 + P, :], ert[:])

            # ---------------- phase E: encoder (node pairs) ----------------
            for pt in range(NT // 2):
                n0 = pt * 2 * P
                obsT = []
                for k in range(2):
                    t = obstp.tile([P, 2 * P], F, tag="obsT")
                    nc.sync.dma_start(t[:], obst_p[k * P:(k + 1) * P,
                                                   n0:n0 + 2 * P])
                    obsT.append(t)
                hT = []
                for m in range(4):
                    ph = pep.tile([P, 2 * P], F32, tag="pe")
                    for k in range(2):
                        nc.tensor.matmul(
                            ph[:], lhsT=W1_sb[k][:, m * P:(m + 1) * P],
                            rhs=obsT[k][:], start=(k == 0), stop=(k == 1))
                    h = encp.tile([P, 2 * P], F, tag=f"h{m}")
                    nc.vector.tensor_scalar(
                        out=h[:], in0=ph[:], scalar1=b1_sb[m][:, 0:1],
                        scalar2=0.0, op0=OP.add, op1=OP.max)
                    hT.append(h)
                pz = pep.tile([P, 2 * P], F32, tag="pe")
                for m in range(4):
                    nc.tensor.matmul(pz[:], lhsT=W2_sb[m][:], rhs=hT[m][:],
                                     start=(m == 0), stop=(m == 3))
                z1T = encp.tile([P, 2 * P], F, tag="z1T")
                nc.vector.tensor_scalar(
                    out=z1T[:], in0=pz[:], scalar1=b2_sb[:, 0:1],
                    scalar2=0.0, op0=OP.add, op1=OP.max)
                for k in range(2):
                    node_tile_products(z1T[:, k * P:(k + 1) * P],
                                       n0 + k * P, 0, 0, None)

            # ---------------- AllGather layer-1 table ----------------
            nc.gpsimd.collective_compute(
                "AllGather", OP.bypass, replica_groups=groups,
                ins=[tab_loc[0][:, :]], outs=[tab_full[0][:, :]])

            # ---------------- edge pass ----------------
            def edge_pass(li, out_col, build_next):
                tabf = tab_full[li]
                ert_d = er_tab[li]
                for D in range(NT):
                    acc = paccp.tile([P, TAB], F32, tag="acc")
                    for t in range(T):
                        ta = D * T + t
                        g = gathp.tile([P, TAB], F, tag="g")
                        nc.gpsimd.indirect_dma_start(
                            out=g[:], out_offset=None, in_=tabf[:, :],
                            in_offset=bass.IndirectOffsetOnAxis(
                                ap=srcs_sb[:, ta:ta + 1], axis=0))
                        er = smallp.tile([P, NH], F, tag="er")
                        nc.gpsimd.indirect_dma_start(
                            out=er[:], out_offset=None, in_=ert_d[:, :],
                            in_offset=bass.IndirectOffsetOnAxis(
                                ap=dloc_sb[:, ta:ta + 1], axis=0))
                        e_t = smallp.tile([P, NH], F, tag="e_t")
                        nc.vector.tensor_add(e_t[:], g[:, H:TAB], er[:])
                        exa = smallp.tile([P, NH], F, tag="exa")
                        nc.scalar.activation(exa[:], e_t[:], AF.Exp, scale=0.2)
                        rhs = rhsp.tile([P, TAB], F, tag="rhs")
                        nc.scalar.activation(rhs[:, H:TAB], e_t[:], AF.Exp)
                        nc.vector.tensor_max(rhs[:, H:TAB], rhs[:, H:TAB],
                                             exa[:])
                        bt = btp.tile([P, P], F, tag="bt")
                        nc.vector.tensor_tensor(
                            out=bt[:],
                            in0=dpos_sb[:, ta:ta + 1].to_broadcast([P, P]),
                            in1=iota_sb[:], op=OP.is_equal)
                        nc.vector.tensor_tensor(
                            out=rhs[:, 0:H].rearrange("p (h d) -> p h d", h=NH),
                            in0=g[:, 0:H].rearrange("p (h d) -> p h d", h=NH),
                            in1=rhs[:, H:TAB].unsqueeze(2).to_broadcast(
                                [P, NH, HD]),
                            op=OP.mult)
                        nc.tensor.matmul(acc[:], lhsT=bt[:], rhs=rhs[:],
                                         start=(t == 0), stop=(t == T - 1))
                    # ---- dst-tile epilogue ----
                    zp = smallp.tile([P, NH], F32, tag="zp")
                    nc.vector.tensor_scalar_add(zp[:], acc[:, H:TAB], 1e-9)
                    zr = smallp.tile([P, NH], F32, tag="zr")
                    nc.vector.reciprocal(zr[:], zp[:])
                    zo = rowsp.tile([P, H], F, tag="zo")
                    nc.vector.scalar_tensor_tensor(
                        out=zo[:].rearrange("p (h d) -> p h d", h=NH),
                        in0=acc[:, 0:H].rearrange("p (h d) -> p h d", h=NH),
                        scalar=0.0,
                        in1=zr[:].unsqueeze(2).to_broadcast([P, NH, HD]),
                        op0=OP.max, op1=OP.mult)
                    nc.sync.dma_start(
                        out_p[D * P:(D + 1) * P, out_col:out_col + H], zo[:])
                    if build_next:
                        pzt = ptrp.tile([P, P], F, tag="ptr")
                        nc.tensor.transpose(pzt[:], zo[:], ident[:])
                        zT = rowsp.tile([P, P], F, tag="zT")
                        nc.vector.tensor_copy(zT[:], pzt[:])
                        # build next-layer table rows for this dst tile
                        pr = prodp.tile([P, H + 2 * NH], F32, tag="pr")
                        nc.tensor.matmul(pr[:, 0:H], lhsT=zT[:],
                                         rhs=Wg_sb[1][:],
                                         start=True, stop=True)
                        nc.tensor.matmul(pr[:, H:H + NH], lhsT=zT[:],
                                         rhs=Wgal_sb[1][:],
                                         start=True, stop=True)
                        nc.tensor.matmul(pr[:, H + NH:H + 2 * NH], lhsT=zT[:],
                                         rhs=Wgar_sb[1][:],
                                         start=True, stop=True)
                        tabt = rowsp.tile([P, TAB], F, tag="tabt")
                        nc.vector.tensor_copy(tabt[:], pr[:, 0:TAB])
                        nc.sync.dma_start(
                            tab_loc[1][D * P:(D + 1) * P, :], tabt[:])
                        ert = rowsp.tile([P, NH], F, tag="ert")
                        nc.vector.tensor_copy(ert[:], pr[:, H + NH:H + 2 * NH])
                        nc.sync.dma_start(
                            er_tab[1][D * P:(D + 1) * P, :], ert[:])

            edge_pass(0, H, True)

            nc.gpsimd.collective_compute(
                "AllGather", OP.bypass, replica_groups=groups,
                ins=[tab_loc[1][:, :]], outs=[tab_full[1][:, :]])

            edge_pass(1, 2 * H, False)

    nc.compile()
    return nc


# ----------------------------------------------------------------------------
# Driver
# ----------------------------------------------------------------------------

def _make_blockdiag(a):
    # a: [NH, HD] -> [H, NH] with bd[h*HD+d, h] = a[h, d]
    bd = np.zeros((H, NH), np.float32)
    for h in range(NH):
        bd[h * HD:(h + 1) * HD, h] = a[h]
    return bd


def run_gnn(inputs, n_tiles_per_core=50, trace=False):
    t_start = time.time()
    obs = np.asarray(inputs["obs"], np.float32)
    src = np.asarray(inputs["src"], np.int64)
    dst = np.asarray(inputs["dst"], np.int64)
    N = obs.shape[0]

    prep = _host_prepare(src, dst, n_tiles_per_core)
    NT, T, NPC, NTOT = n_tiles_per_core, prep["T"], prep["NPC"], prep["NTOT"]
    perm = prep["perm"]

    al1bd = _make_blockdiag(np.asarray(inputs["al1"], np.float32))
    ar1bd = _make_blockdiag(np.asarray(inputs["ar1"], np.float32))
    al2bd = _make_blockdiag(np.asarray(inputs["al2"], np.float32))
    ar2bd = _make_blockdiag(np.asarray(inputs["ar2"], np.float32))
    Wg1 = np.asarray(inputs["Wg1"], np.float32)
    Wg2 = np.asarray(inputs["Wg2"], np.float32)
    shared = {
        "w1": np.asarray(inputs["W1"], np.float32),
        "b1": np.asarray(inputs["b1"], np.float32).reshape(HID, 1),
        "w2": np.asarray(inputs["W2"], np.float32),
        "b2": np.asarray(inputs["b2"], np.float32).reshape(H, 1),
        "wg1": Wg1, "wg2": Wg2,
        "wgal1": (Wg1 @ al1bd).astype(np.float32),
        "wgar1": (Wg1 @ ar1bd).astype(np.float32),
        "wgal2": (Wg2 @ al2bd).astype(np.float32),
        "wgar2": (Wg2 @ ar2bd).astype(np.float32),
        "iota": np.tile(np.arange(P, dtype=np.float32)[None, :], (P, 1)),
        "iotac": np.arange(P, dtype=np.float32).reshape(P, 1),
    }

    obs_pad = np.zeros((NTOT, OBS_D), np.float32)
    obs_pad[:N] = obs
    obs_perm = obs_pad[perm]

    in_maps = []
    for c in range(NCORES):
        m = dict(shared)
        m["obst"] = np.ascontiguousarray(
            obs_perm[c * NPC:(c + 1) * NPC].T)
        m["srcslab"] = prep["srcix"][c]
        m["dlocslab"] = prep["dlocal"][c]
        m["dposslab"] = prep["dposf"][c]
        in_maps.append(m)

    t_prep = time.time()
    nc = _build_program(NT, T)
    t_build = time.time()

    from concourse.bass_utils import run_bass_kernel_spmd
    if trace:
        _ensure_ntff_hook()
    res = run_bass_kernel_spmd(nc, in_maps, core_ids=list(range(NCORES)),
                               trace=trace)
    t_run = time.time()

    full = np.concatenate([res.results[c]["out"] for c in range(NCORES)],
                          axis=0)  # [NTOT, 384] permuted order
    out = np.empty((N, 3 * H), np.float32)
    keep = perm < N
    out[perm[keep]] = full[keep]

    LAST_INFO.clear()
    LAST_INFO.update(dict(
        exec_time_ns=res.exec_time_ns, T=T,
        binload_max=int(prep["binload"].max()),
        binload_mean=float(prep["binload"].mean()),
        t_prep=t_prep - t_start, t_build=t_build - t_prep,
        t_run=t_run - t_build,
        profile_json=getattr(res, "profile_json", None),
    ))
    return out


def kernel(**inputs):
    return run_gnn(inputs, n_tiles_per_core=50,
                   trace=bool(os.environ.get("GNN_TRACE")))


# revision 13
# speedup vs baseline: 1.1787x; 1.1787x over previous
"""Distributed multi-head GAT (encoder + 2 GAT layers) on 8 TRN2 NeuronCores.

Strategy (graph/data parallel, per the dst-ownership sharding):
  * Nodes are permuted and dealt into 8*NT bins of 128 nodes so that every
    bin (= one PSUM dst-tile) has a near-equal number of incoming edges and
    every core has a near-equal total.  Edges live with the core that owns
    their dst node.
  * Each core encodes its own node shard (obs -> z1), builds a per-node
    bf16 table row [feat(128) | el(8) | pad] (512B stride for dma_gather),
    and the 8 shards are AllGathered into a full table so any core can
    gather src rows (the "halo" of a random graph is the full table).
  * Edge pass: each bin's edges are split into low-src (table row < 32768)
    and high-src tile groups so the batched int16 `dma_gather` can address
    the table; one gather per super-tile per group + one local er gather.
    Batched DVE ops compute ex = exp(leaky_relu(el+er)) (exactly
    max(exp(x), exp(0.2x))) and scale messages; per 128-edge tile a one-hot
    matmul (Bt[e,d] = [dstpos[e] == d]) reduces into the bin's PSUM
    accumulator.  The softmax max-subtraction is skipped: inputs are O(0.1)
    so exp is safe, and the result matches up to the 1e-9 epsilon scaling.
  * Epilogue per bin: out = relu(acc_feat) * 1/(acc_z + 1e-9) per head;
    also builds the next layer's table row + er entries.
"""

import os
import sys
import time

import numpy as np

for _p in ("/opt/trn_rl_repo", "/root/.axon_site/_ro/trn_rl_repo"):
    if os.path.isdir(_p) and _p not in sys.path:
        sys.path.insert(0, _p)

P = 128
NCORES = 8
OBS_D = 256
HID = 512
H = 128          # h_dim
NH = 8           # heads
HD = 16          # head dim
TABW = 256       # table row width (bf16) -> 512B stride; cols 0:136 used
TAB = H + NH     # useful table row: feat | el
ERW = 128        # er table row width (bf16) -> 256B stride; cols 0:8 used
SUP = 2          # bins per gather super-tile
SPLIT = 32768    # low/high table split for int16 gather indices
PAD_SENTINEL = 200.0

LAST_INFO = {}


def _ensure_ntff_hook():
    """Register the axon NTFF profile hook if the image's antenv lacks it."""
    try:
        import types

        import antenv
        try:
            from antenv import axon_hooks  # noqa: F401
            return
        except ImportError:
            pass
        m = types.ModuleType("antenv.axon_hooks")
        _h = [None]
        m.set_axon_ntff_profile_hook = lambda hook: _h.__setitem__(0, hook)
        m.get_axon_ntff_profile_hook = lambda: _h[0]
        sys.modules["antenv.axon_hooks"] = m
        antenv.axon_hooks = m
        from trn_agent_boot.trn_boot import _ntff_profile_via_ctypes
        m.set_axon_ntff_profile_hook(
            _ntff_profile_via_ctypes("/opt/axon/libaxon_pjrt.so"))
    except Exception as e:  # profiling is best-effort
        print(f"ntff hook setup failed: {e}")


# ----------------------------------------------------------------------------
# Host-side preprocessing
# ----------------------------------------------------------------------------

def _wrap16(a):
    """[n] -> [128, n/16] int16 in the dma_gather wrapped layout:
    index j lives at partition j%16, col j//16, replicated to all 8 groups."""
    n = a.shape[0]
    w = a.reshape(n // 16, 16).T.astype(np.int16)    # [16, n/16]
    return np.ascontiguousarray(np.tile(w, (8, 1)))


def _host_prepare(src, dst, n_tiles_per_core, split):
    """Balance bins, split edges by src table half, build gather slabs."""
    import ml_dtypes

    NT = n_tiles_per_core
    NPC = NT * P
    NTOT = NCORES * NPC
    NBINS = NCORES * NT
    E = src.shape[0]

    deg = np.bincount(dst, minlength=NTOT).astype(np.int64)
    order = np.argsort(-deg, kind="stable")
    arr = order.reshape(P, NBINS).copy()
    arr[1::2] = arr[1::2, ::-1]
    perm = arr.T.reshape(-1)                       # g -> orig node
    pos = np.empty(NTOT, np.int64)
    pos[perm] = np.arange(NTOT)                    # orig node -> g

    srcg = pos[src]
    dstg = pos[dst]
    binid = dstg // P
    low = srcg < split

    nlo = np.bincount(binid[low], minlength=NBINS)
    nhi = np.bincount(binid[~low], minlength=NBINS)
    TL = max(1, int(np.ceil(nlo.max() / P)))
    TH = max(1, int(np.ceil(nhi.max() / P)))
    TT = TL + TH

    gkey = binid * 2 + (~low).astype(np.int64)
    eorder = np.argsort(gkey, kind="stable")
    counts = np.bincount(gkey, minlength=2 * NBINS)
    starts = np.concatenate([[0], np.cumsum(counts)[:-1]])
    rank = np.arange(E) - starts[gkey[eorder]]
    ghigh = gkey[eorder] % 2
    slot = (gkey[eorder] // 2) * (TT * P) + ghigh * (TL * P) + rank

    ES = NBINS * TT * P
    sg = np.zeros(ES, np.int64)
    isreal = np.zeros(ES, bool)
    dlocal = np.zeros(ES, np.int64)
    dposf = np.full(ES, PAD_SENTINEL, np.float32)
    sg[slot] = srcg[eorder]
    isreal[slot] = True
    dlocal[slot] = dstg[eorder] % NPC
    dposf[slot] = (dstg[eorder] % P).astype(np.float32)

    # high-section pads must index within the high table half
    sg[~isreal & (np.arange(ES) % (TT * P) >= TL * P)] = split

    sg = sg.reshape(NCORES, NT, TT * P)
    dlocal = dlocal.reshape(NCORES, NT, TT * P)
    dposf = dposf.reshape(NCORES, NT, TT * P).astype(ml_dtypes.bfloat16)

    NSUP = NT // SUP
    idxlow, idxhigh, idxer, dpos = [], [], [], []
    for c in range(NCORES):
        il, ih, ie, dp = [], [], [], []
        for S in range(NSUP):
            b0 = S * SUP
            lo_slots = sg[c, b0:b0 + SUP, :TL * P].reshape(-1)
            hi_slots = sg[c, b0:b0 + SUP, TL * P:].reshape(-1) - split
            er_slots = np.concatenate([
                dlocal[c, b0:b0 + SUP, :TL * P].reshape(-1),
                dlocal[c, b0:b0 + SUP, TL * P:].reshape(-1)])
            il.append(_wrap16(lo_slots))
            ih.append(_wrap16(hi_slots))
            ie.append(_wrap16(er_slots))
            dpl = dposf[c, b0:b0 + SUP, :TL * P].reshape(SUP * TL, P)
            dph = dposf[c, b0:b0 + SUP, TL * P:].reshape(SUP * TH, P)
            dp.append(np.concatenate([dpl, dph], 0).T)   # [128, SUP*TT]
        idxlow.append(np.ascontiguousarray(np.concatenate(il, 1)))
        idxhigh.append(np.ascontiguousarray(np.concatenate(ih, 1)))
        idxer.append(np.ascontiguousarray(np.concatenate(ie, 1)))
        dpos.append(np.ascontiguousarray(
            np.concatenate(dp, 1).astype(ml_dtypes.bfloat16)))

    binload = deg[arr].sum(axis=0)
    return dict(
        perm=perm, TL=TL, TH=TH, NPC=NPC, NTOT=NTOT,
        idxlow=idxlow, idxhigh=idxhigh, idxer=idxer, dpos=dpos,
        binload=binload,
    )


# ----------------------------------------------------------------------------
# Device program
# ----------------------------------------------------------------------------

def _build_program(NT, TL, TH, split):
    import concourse.bacc as bacc
    import concourse.mybir as mybir
    import concourse.tile as tile

    dt = mybir.dt
    F = dt.float32r      # 4-byte float, fast PE mode
    F32 = dt.float32
    BF = dt.bfloat16
    I16 = dt.int16
    AF = mybir.ActivationFunctionType
    OP = mybir.AluOpType

    NPC = NT * P
    NTOT = NCORES * NPC
    TT = TL + TH
    assert NT % SUP == 0
    NSUP = NT // SUP
    NLO = SUP * TL * P       # low slots per super
    NHI = SUP * TH * P
    CL, CH, CE = NLO // 16, NHI // 16, (NLO + NHI) // 16

    nc = bacc.Bacc("TRN2", target_bir_lowering=False, debug=False,
                   num_devices=NCORES)

    obst_p = nc.dram_tensor("obst", [OBS_D, NPC], F, kind="ExternalInput")
    W1_p = nc.dram_tensor("w1", [OBS_D, HID], F, kind="ExternalInput")
    b1_p = nc.dram_tensor("b1", [HID, 1], F32, kind="ExternalInput")
    W2_p = nc.dram_tensor("w2", [HID, H], F, kind="ExternalInput")
    b2_p = nc.dram_tensor("b2", [H, 1], F32, kind="ExternalInput")
    Wg_p = [nc.dram_tensor(f"wg{i}", [H, H], BF, kind="ExternalInput")
            for i in (1, 2)]
    Wgal_p = [nc.dram_tensor(f"wgal{i}", [H, NH], BF, kind="ExternalInput")
              for i in (1, 2)]
    Wgar_p = [nc.dram_tensor(f"wgar{i}", [H, NH], BF, kind="ExternalInput")
              for i in (1, 2)]
    iota_p = nc.dram_tensor("iota", [P, P], BF, kind="ExternalInput")
    identf_p = nc.dram_tensor("identf", [P, P], F, kind="ExternalInput")
    il_p = nc.dram_tensor("idxlow", [P, NSUP * CL], I16, kind="ExternalInput")
    ih_p = nc.dram_tensor("idxhigh", [P, NSUP * CH], I16, kind="ExternalInput")
    ie_p = nc.dram_tensor("idxer", [P, NSUP * CE], I16, kind="ExternalInput")
    dpos_p = nc.dram_tensor("dposslab", [P, NT * TT], BF, kind="ExternalInput")
    out_p = nc.dram_tensor("out", [NPC, 3 * H], F, kind="ExternalOutput")

    tab_loc = [nc.dram_tensor(f"tab{i}_loc", [NPC, TABW], BF) for i in (1, 2)]
    tab_full = [nc.dram_tensor(f"tab{i}_full", [NTOT, TABW], BF,
                               addr_space="Shared") for i in (1, 2)]
    er_tab = [nc.dram_tensor(f"er{i}_tab", [NPC, ERW], BF) for i in (1, 2)]

    groups = [list(range(NCORES))]

    with tile.TileContext(nc) as tc:
        with (
            tc.tile_pool(name="const", bufs=1) as constp,
            tc.tile_pool(name="obst", bufs=2) as obstp,
            tc.tile_pool(name="enc", bufs=2) as encp,
            tc.tile_pool(name="rows", bufs=3) as rowsp,
            tc.tile_pool(name="idx", bufs=3) as idxp,
            tc.tile_pool(name="gath", bufs=3) as gathp,
            tc.tile_pool(name="small", bufs=3) as smallp,
            tc.tile_pool(name="rhs", bufs=3) as rhsp,
            tc.tile_pool(name="bt", bufs=3) as btp,
            tc.tile_pool(name="pe", bufs=2, space="PSUM") as pep,
            tc.tile_pool(name="pacc", bufs=3, space="PSUM") as paccp,
            tc.tile_pool(name="ptr", bufs=1, space="PSUM") as ptrp,
            tc.tile_pool(name="prod", bufs=2, space="PSUM") as prodp,
        ):
            # ---------------- prologue ----------------
            iota_sb = constp.tile([P, P], BF, tag="iota")
            nc.sync.dma_start(iota_sb[:], iota_p[:, :])
            ident = constp.tile([P, P], F, tag="ident")
            nc.sync.dma_start(ident[:], identf_p[:, :])

            W1_sb = []
            for k in range(2):
                t = constp.tile([P, HID], F, tag=f"w1_{k}")
                nc.sync.dma_start(t[:], W1_p[k * P:(k + 1) * P, :])
                W1_sb.append(t)
            W2_sb = []
            for m in range(4):
                t = constp.tile([P, H], F, tag=f"w2_{m}")
                nc.sync.dma_start(t[:], W2_p[m * P:(m + 1) * P, :])
                W2_sb.append(t)
            b1_sb = []
            for m in range(4):
                t = constp.tile([P, 1], F32, tag=f"b1_{m}")
                nc.sync.dma_start(t[:], b1_p[m * P:(m + 1) * P, :])
                b1_sb.append(t)
            b2_sb = constp.tile([P, 1], F32, tag="b2")
            nc.sync.dma_start(b2_sb[:], b2_p[:, :])
            Wg_sb, Wgal_sb, Wgar_sb = [], [], []
            for i in range(2):
                t = constp.tile([P, H], BF, tag=f"wg_{i}")
                nc.sync.dma_start(t[:], Wg_p[i][:, :])
                Wg_sb.append(t)
                t = constp.tile([P, NH], BF, tag=f"wgal_{i}")
                nc.sync.dma_start(t[:], Wgal_p[i][:, :])
                Wgal_sb.append(t)
                t = constp.tile([P, NH], BF, tag=f"wgar_{i}")
                nc.sync.dma_start(t[:], Wgar_p[i][:, :])
                Wgar_sb.append(t)

            def table_products(zTb_chunk, row0, li):
                pr = prodp.tile([P, H + 2 * NH], F32, tag="pr")
                nc.tensor.matmul(pr[:, 0:H], lhsT=zTb_chunk, rhs=Wg_sb[li][:],
                                 start=True, stop=True)
                nc.tensor.matmul(pr[:, H:H + NH], lhsT=zTb_chunk,
                                 rhs=Wgal_sb[li][:], start=True, stop=True)
                nc.tensor.matmul(pr[:, H + NH:H + 2 * NH], lhsT=zTb_chunk,
                                 rhs=Wgar_sb[li][:], start=True, stop=True)
                tabt = rowsp.tile([P, TAB], BF, tag="tabt")
                nc.vector.tensor_copy(tabt[:], pr[:, 0:TAB])
                nc.sync.dma_start(tab_loc[li][row0:row0 + P, 0:TAB], tabt[:])
                ert = rowsp.tile([P, NH], BF, tag="ert")
                nc.vector.tensor_copy(ert[:], pr[:, H + NH:H + 2 * NH])
                nc.sync.dma_start(er_tab[li][row0:row0 + P, 0:NH], ert[:])

            # ---------------- phase E: encoder ----------------
            for pt in range(NT // 2):
                n0 = pt * 2 * P
                obsT = []
                for k in range(2):
                    t = obstp.tile([P, 2 * P], F, tag="obsT")
                    nc.sync.dma_start(t[:], obst_p[k * P:(k + 1) * P,
                                                   n0:n0 + 2 * P])
                    obsT.append(t)
                hT = []
                for m in range(4):
                    ph = pep.tile([P, 2 * P], F32, tag="pe")
                    for k in range(2):
                        nc.tensor.matmul(
                            ph[:], lhsT=W1_sb[k][:, m * P:(m + 1) * P],
                            rhs=obsT[k][:], start=(k == 0), stop=(k == 1))
                    h = encp.tile([P, 2 * P], F, tag=f"h{m}")
                    nc.vector.tensor_scalar(
                        out=h[:], in0=ph[:], scalar1=b1_sb[m][:, 0:1],
                        scalar2=0.0, op0=OP.add, op1=OP.max)
                    hT.append(h)
                pz = pep.tile([P, 2 * P], F32, tag="pe")
                for m in range(4):
                    nc.tensor.matmul(pz[:], lhsT=W2_sb[m][:], rhs=hT[m][:],
                                     start=(m == 0), stop=(m == 3))
                z1T = encp.tile([P, 2 * P], F, tag="z1T")
                nc.vector.tensor_scalar(
                    out=z1T[:], in0=pz[:], scalar1=b2_sb[:, 0:1],
                    scalar2=0.0, op0=OP.add, op1=OP.max)
                z1Tb = encp.tile([P, 2 * P], BF, tag="z1Tb")
                nc.vector.tensor_copy(z1Tb[:], z1T[:])
                for k in range(2):
                    ptr = ptrp.tile([P, P], F, tag="ptr")
                    nc.tensor.transpose(ptr[:], z1T[:, k * P:(k + 1) * P],
                                        ident[:])
                    zrow = rowsp.tile([P, P], F, tag="zrows")
                    nc.vector.tensor_copy(zrow[:], ptr[:])
                    nc.sync.dma_start(
                        out_p[n0 + k * P:n0 + (k + 1) * P, 0:H], zrow[:])
                    table_products(z1Tb[:, k * P:(k + 1) * P], n0 + k * P, 0)

            nc.gpsimd.collective_compute(
                "AllGather", OP.bypass, replica_groups=groups,
                ins=[tab_loc[0][:, :]], outs=[tab_full[0][:, :]])

            # ---------------- edge pass ----------------
            def epilogue(D, acc, out_col, build_next):
                zp = smallp.tile([P, NH], F32, tag="zp")
                nc.vector.tensor_scalar_add(zp[:], acc[:, H:TAB], 1e-9)
                zrec = smallp.tile([P, NH], F32, tag="zrec")
                nc.vector.reciprocal(zrec[:], zp[:])
                zo = rowsp.tile([P, H], F, tag="zo")
                nc.vector.scalar_tensor_tensor(
                    out=zo[:].rearrange("p (h d) -> p h d", h=NH),
                    in0=acc[:, 0:H].rearrange("p (h d) -> p h d", h=NH),
                    scalar=0.0,
                    in1=zrec[:].unsqueeze(2).to_broadcast([P, NH, HD]),
                    op0=OP.max, op1=OP.mult)
                nc.sync.dma_start(
                    out_p[D * P:(D + 1) * P, out_col:out_col + H], zo[:])
                if build_next:
                    pzt = ptrp.tile([P, P], F, tag="ptr")
                    nc.tensor.transpose(pzt[:], zo[:], ident[:])
                    zTb = rowsp.tile([P, P], BF, tag="zTb")
                    nc.vector.tensor_copy(zTb[:], pzt[:])
                    table_products(zTb[:], D * P, 1)

            def edge_block(g, erc, er_off, dpos_sb, dp_off, accs,
                           S, TB, out_col, build_next, is_high):
                """One low/high block of a super: batched attn + per-tile mm."""
                n = SUP * TB
                g3 = g[:].rearrange("p (c e) -> p c e", e=TABW)
                e_t = smallp.tile([P, n * NH], F32,
                                  tag="e_th" if is_high else "e_tl")
                nc.vector.tensor_add(
                    e_t[:].rearrange("p (c e) -> p c e", e=NH),
                    g3[:, :, H:TAB],
                    erc[:].rearrange("p (c e) -> p c e", e=NH)
                       [:, er_off:er_off + n, :])
                ex1 = smallp.tile([P, n * NH], F32,
                                  tag="ex1h" if is_high else "ex1l")
                nc.scalar.activation(ex1[:], e_t[:], AF.Exp)
                ex2 = smallp.tile([P, n * NH], F32,
                                  tag="ex2h" if is_high else "ex2l")
                nc.scalar.activation(ex2[:], e_t[:], AF.Exp, scale=0.2)
                rhs = rhsp.tile([P, n * TAB], BF,
                                tag="rhsh" if is_high else "rhsl")
                r3 = rhs[:].rearrange("p (c e) -> p c e", e=TAB)
                nc.vector.tensor_max(
                    r3[:, :, H:TAB],
                    ex1[:].rearrange("p (c e) -> p c e", e=NH),
                    ex2[:].rearrange("p (c e) -> p c e", e=NH))
                bt = btp.tile([P, n * P], BF,
                              tag="bth" if is_high else "btl")
                nc.vector.tensor_tensor(
                    out=bt[:].rearrange("p (c e) -> p c e", e=P),
                    in0=dpos_sb[:, dp_off:dp_off + n].unsqueeze(2)
                        .to_broadcast([P, n, P]),
                    in1=iota_sb[:].unsqueeze(1).to_broadcast([P, n, P]),
                    op=OP.is_equal)
                nc.vector.tensor_tensor(
                    out=r3[:, :, 0:H].rearrange("p c (h d) -> p c h d", h=NH),
                    in0=g3[:, :, 0:H].rearrange("p c (h d) -> p c h d", h=NH),
                    in1=r3[:, :, H:TAB].unsqueeze(3)
                        .to_broadcast([P, n, NH, HD]),
                    op=OP.mult)
                for c in range(n):
                    b = c // TB
                    t = c % TB
                    D = S * SUP + b
                    if not is_high and t == 0:
                        acc_new = paccp.tile([P, TAB], F32, tag="acc")
                        accs[b] = acc_new
                    nc.tensor.matmul(
                        accs[b][:], lhsT=bt[:, c * P:(c + 1) * P],
                        rhs=r3[:, c, :],
                        start=(not is_high and t == 0),
                        stop=(is_high and t == TB - 1))
                    if is_high and t == TB - 1:
                        epilogue(D, accs[b], out_col, build_next)

            def edge_pass(li, out_col, build_next):
                tabf = tab_full[li]
                ert_d = er_tab[li]
                accs = [None] * SUP
                for S in range(NSUP):
                    ilow = idxp.tile([P, CL], I16, tag="ilow")
                    nc.sync.dma_start(ilow[:], il_p[:, S * CL:(S + 1) * CL])
                    ihigh = idxp.tile([P, CH], I16, tag="ihigh")
                    nc.sync.dma_start(ihigh[:], ih_p[:, S * CH:(S + 1) * CH])
                    ier = idxp.tile([P, CE], I16, tag="ier")
                    nc.sync.dma_start(ier[:], ie_p[:, S * CE:(S + 1) * CE])
                    dpos_sb = idxp.tile([P, SUP * TT], BF, tag="dpos")
                    nc.sync.dma_start(
                        dpos_sb[:],
                        dpos_p[:, S * SUP * TT:(S + 1) * SUP * TT])

                    glow = gathp.tile([P, SUP * TL * TABW], BF, tag="glow")
                    nc.gpsimd.dma_gather(
                        out_ap=glow[:].rearrange("p (c e) -> p c e", e=TABW),
                        in_ap=tabf[:, :], idxs_ap=ilow[:],
                        num_idxs=NLO, num_idxs_reg=NLO, elem_size=TABW,
                        single_packet=False)
                    ghigh = gathp.tile([P, SUP * TH * TABW], BF, tag="ghigh")
                    nc.gpsimd.dma_gather(
                        out_ap=ghigh[:].rearrange("p (c e) -> p c e", e=TABW),
                        in_ap=tabf[split:NTOT, :], idxs_ap=ihigh[:],
                        num_idxs=NHI, num_idxs_reg=NHI, elem_size=TABW,
                        single_packet=False)
                    erg = gathp.tile([P, SUP * TT * ERW], BF, tag="erg")
                    nc.gpsimd.dma_gather(
                        out_ap=erg[:].rearrange("p (c e) -> p c e", e=ERW),
                        in_ap=ert_d[:, :], idxs_ap=ier[:],
                        num_idxs=NLO + NHI, num_idxs_reg=NLO + NHI,
                        elem_size=ERW, single_packet=False)
                    # compact er columns (0:8 of each 128-wide row)
                    erc = smallp.tile([P, SUP * TT * NH], BF, tag="erc")
                    nc.vector.tensor_copy(
                        erc[:].rearrange("p (c e) -> p c e", e=NH),
                        erg[:].rearrange("p (c e) -> p c e", e=ERW)[:, :, 0:NH])
                    edge_block(glow, erc, 0, dpos_sb, 0, accs,
                               S, TL, out_col, build_next, False)
                    edge_block(ghigh, erc, SUP * TL, dpos_sb,
                               SUP * TL, accs, S, TH, out_col, build_next,
                               True)

            edge_pass(0, H, True)

            nc.gpsimd.collective_compute(
                "AllGather", OP.bypass, replica_groups=groups,
                ins=[tab_loc[1][:, :]], outs=[tab_full[1][:, :]])

            edge_pass(1, 2 * H, False)

    nc.compile()
    return nc


# ----------------------------------------------------------------------------
# Driver
# ----------------------------------------------------------------------------

def _make_blockdiag(a):
    bd = np.zeros((H, NH), np.float32)
    for h in range(NH):
        bd[h * HD:(h + 1) * HD, h] = a[h]
    return bd


def run_gnn(inputs, n_tiles_per_core=52, trace=False):
    import ml_dtypes
    bf16 = ml_dtypes.bfloat16

    t_start = time.time()
    obs = np.asarray(inputs["obs"], np.float32)
    src = np.asarray(inputs["src"], np.int64)
    dst = np.asarray(inputs["dst"], np.int64)
    N = obs.shape[0]

    NTOT_ = NCORES * n_tiles_per_core * P
    split = min(SPLIT, NTOT_ // 2)
    prep = _host_prepare(src, dst, n_tiles_per_core, split)
    NT = n_tiles_per_core
    TL, TH, NPC, NTOT = prep["TL"], prep["TH"], prep["NPC"], prep["NTOT"]
    perm = prep["perm"]

    al1bd = _make_blockdiag(np.asarray(inputs["al1"], np.float32))
    ar1bd = _make_blockdiag(np.asarray(inputs["ar1"], np.float32))
    al2bd = _make_blockdiag(np.asarray(inputs["al2"], np.float32))
    ar2bd = _make_blockdiag(np.asarray(inputs["ar2"], np.float32))
    Wg1 = np.asarray(inputs["Wg1"], np.float32)
    Wg2 = np.asarray(inputs["Wg2"], np.float32)
    shared = {
        "w1": np.asarray(inputs["W1"], np.float32),
        "b1": np.asarray(inputs["b1"], np.float32).reshape(HID, 1),
        "w2": np.asarray(inputs["W2"], np.float32),
        "b2": np.asarray(inputs["b2"], np.float32).reshape(H, 1),
        "wg1": Wg1.astype(bf16), "wg2": Wg2.astype(bf16),
        "wgal1": (Wg1 @ al1bd).astype(bf16),
        "wgar1": (Wg1 @ ar1bd).astype(bf16),
        "wgal2": (Wg2 @ al2bd).astype(bf16),
        "wgar2": (Wg2 @ ar2bd).astype(bf16),
        "iota": np.tile(np.arange(P, dtype=np.float32)[None, :],
                        (P, 1)).astype(bf16),
        "identf": np.eye(P, dtype=np.float32),
    }

    obs_pad = np.zeros((NTOT, OBS_D), np.float32)
    obs_pad[:N] = obs
    obs_perm = obs_pad[perm]

    in_maps = []
    for c in range(NCORES):
        m = dict(shared)
        m["obst"] = np.ascontiguousarray(obs_perm[c * NPC:(c + 1) * NPC].T)
        m["idxlow"] = prep["idxlow"][c]
        m["idxhigh"] = prep["idxhigh"][c]
        m["idxer"] = prep["idxer"][c]
        m["dposslab"] = prep["dpos"][c]
        in_maps.append(m)

    t_prep = time.time()
    nc = _build_program(NT, TL, TH, split)
    t_build = time.time()

    from concourse.bass_utils import run_bass_kernel_spmd
    if trace:
        _ensure_ntff_hook()
    res = run_bass_kernel_spmd(nc, in_maps, core_ids=list(range(NCORES)),
                               trace=trace)
    t_run = time.time()

    full = np.concatenate([res.results[c]["out"] for c in range(NCORES)],
                          axis=0)
    out = np.empty((N, 3 * H), np.float32)
    keep = perm < N
    out[perm[keep]] = full[keep]

    LAST_INFO.clear()
    LAST_INFO.update(dict(
        exec_time_ns=res.exec_time_ns, TL=TL, TH=TH,
        binload_max=int(prep["binload"].max()),
        t_prep=t_prep - t_start, t_build=t_build - t_prep,
        t_run=t_run - t_build,
        profile_json=getattr(res, "profile_json", None),
    ))
    return out


def kernel(**inputs):
    return run_gnn(inputs, n_tiles_per_core=52,
                   trace=bool(os.environ.get("GNN_TRACE")))


# revision 14
# speedup vs baseline: 1.1834x; 1.0040x over previous
"""Distributed multi-head GAT (encoder + 2 GAT layers) on 8 TRN2 NeuronCores.

Strategy (graph/data parallel, per the dst-ownership sharding):
  * Nodes are permuted and dealt into 8*NT bins of 128 nodes so that every
    bin (= one PSUM dst-tile) has a near-equal number of incoming edges and
    every core has a near-equal total.  Edges live with the core that owns
    their dst node.
  * Each core encodes its own node shard (obs -> z1), builds a per-node
    bf16 table row [feat(128) | el(8) | pad] (512B stride for dma_gather),
    and the 8 shards are AllGathered into a full table so any core can
    gather src rows (the "halo" of a random graph is the full table).
  * Edge pass: each bin's edges are split into low-src (table row < 32768)
    and high-src tile groups so the batched int16 `dma_gather` can address
    the table; one gather per super-tile per group + one local er gather.
    Batched DVE ops compute ex = exp(leaky_relu(el+er)) (exactly
    max(exp(x), exp(0.2x))) and scale messages; per 128-edge tile a one-hot
    matmul (Bt[e,d] = [dstpos[e] == d]) reduces into the bin's PSUM
    accumulator.  The softmax max-subtraction is skipped: inputs are O(0.1)
    so exp is safe, and the result matches up to the 1e-9 epsilon scaling.
  * Epilogue per bin: out = relu(acc_feat) * 1/(acc_z + 1e-9) per head;
    also builds the next layer's table row + er entries.
"""

import os
import sys
import time

import numpy as np

for _p in ("/opt/trn_rl_repo", "/root/.axon_site/_ro/trn_rl_repo"):
    if os.path.isdir(_p) and _p not in sys.path:
        sys.path.insert(0, _p)

P = 128
NCORES = 8
OBS_D = 256
HID = 512
H = 128          # h_dim
NH = 8           # heads
HD = 16          # head dim
TABW = 256       # table row width (bf16) -> 512B stride; cols 0:136 used
TAB = H + NH     # useful table row: feat | el
ERW = 128        # er table row width (bf16) -> 256B stride; cols 0:8 used
SUP = 2          # bins per gather super-tile
SPLIT = 32768    # low/high table split for int16 gather indices
PAD_SENTINEL = 200.0

LAST_INFO = {}


def _ensure_ntff_hook():
    """Register the axon NTFF profile hook if the image's antenv lacks it."""
    try:
        import types

        import antenv
        try:
            from antenv import axon_hooks  # noqa: F401
            return
        except ImportError:
            pass
        m = types.ModuleType("antenv.axon_hooks")
        _h = [None]
        m.set_axon_ntff_profile_hook = lambda hook: _h.__setitem__(0, hook)
        m.get_axon_ntff_profile_hook = lambda: _h[0]
        sys.modules["antenv.axon_hooks"] = m
        antenv.axon_hooks = m
        from trn_agent_boot.trn_boot import _ntff_profile_via_ctypes
        m.set_axon_ntff_profile_hook(
            _ntff_profile_via_ctypes("/opt/axon/libaxon_pjrt.so"))
    except Exception as e:  # profiling is best-effort
        print(f"ntff hook setup failed: {e}")


# ----------------------------------------------------------------------------
# Host-side preprocessing
# ----------------------------------------------------------------------------

def _wrap16(a):
    """[n] -> [128, n/16] int16 in the dma_gather wrapped layout:
    index j lives at partition j%16, col j//16, replicated to all 8 groups."""
    n = a.shape[0]
    w = a.reshape(n // 16, 16).T.astype(np.int16)    # [16, n/16]
    return np.ascontiguousarray(np.tile(w, (8, 1)))


def _host_prepare(src, dst, n_tiles_per_core, split):
    """Balance bins, split edges by src table half, build gather slabs."""
    import ml_dtypes

    NT = n_tiles_per_core
    NPC = NT * P
    NTOT = NCORES * NPC
    NBINS = NCORES * NT
    E = src.shape[0]

    deg = np.bincount(dst, minlength=NTOT).astype(np.int64)
    order = np.argsort(-deg, kind="stable")
    arr = order.reshape(P, NBINS).copy()
    arr[1::2] = arr[1::2, ::-1]
    perm = arr.T.reshape(-1)                       # g -> orig node
    pos = np.empty(NTOT, np.int64)
    pos[perm] = np.arange(NTOT)                    # orig node -> g

    srcg = pos[src]
    dstg = pos[dst]
    binid = dstg // P
    low = srcg < split

    nlo = np.bincount(binid[low], minlength=NBINS)
    nhi = np.bincount(binid[~low], minlength=NBINS)
    TL = max(1, int(np.ceil(nlo.max() / P)))
    TH = max(1, int(np.ceil(nhi.max() / P)))
    TT = TL + TH

    gkey = binid * 2 + (~low).astype(np.int64)
    eorder = np.argsort(gkey, kind="stable")
    counts = np.bincount(gkey, minlength=2 * NBINS)
    starts = np.concatenate([[0], np.cumsum(counts)[:-1]])
    rank = np.arange(E) - starts[gkey[eorder]]
    ghigh = gkey[eorder] % 2
    slot = (gkey[eorder] // 2) * (TT * P) + ghigh * (TL * P) + rank

    ES = NBINS * TT * P
    sg = np.zeros(ES, np.int64)
    isreal = np.zeros(ES, bool)
    dlocal = np.zeros(ES, np.int64)
    dposf = np.full(ES, PAD_SENTINEL, np.float32)
    sg[slot] = srcg[eorder]
    isreal[slot] = True
    dlocal[slot] = dstg[eorder] % NPC
    dposf[slot] = (dstg[eorder] % P).astype(np.float32)

    # high-section pads must index within the high table half
    sg[~isreal & (np.arange(ES) % (TT * P) >= TL * P)] = split

    sg = sg.reshape(NCORES, NT, TT * P)
    dlocal = dlocal.reshape(NCORES, NT, TT * P)
    dposf = dposf.reshape(NCORES, NT, TT * P).astype(ml_dtypes.bfloat16)

    NSUP = NT // SUP
    idxlow, idxhigh, idxer, dpos = [], [], [], []
    for c in range(NCORES):
        il, ih, ie, dp = [], [], [], []
        for S in range(NSUP):
            b0 = S * SUP
            lo_slots = sg[c, b0:b0 + SUP, :TL * P].reshape(-1)
            hi_slots = sg[c, b0:b0 + SUP, TL * P:].reshape(-1) - split
            er_slots = np.concatenate([
                dlocal[c, b0:b0 + SUP, :TL * P].reshape(-1),
                dlocal[c, b0:b0 + SUP, TL * P:].reshape(-1)])
            il.append(_wrap16(lo_slots))
            ih.append(_wrap16(hi_slots))
            ie.append(_wrap16(er_slots))
            dpl = dposf[c, b0:b0 + SUP, :TL * P].reshape(SUP * TL, P)
            dph = dposf[c, b0:b0 + SUP, TL * P:].reshape(SUP * TH, P)
            dp.append(np.concatenate([dpl, dph], 0).T)   # [128, SUP*TT]
        idxlow.append(np.ascontiguousarray(np.concatenate(il, 1)))
        idxhigh.append(np.ascontiguousarray(np.concatenate(ih, 1)))
        idxer.append(np.ascontiguousarray(np.concatenate(ie, 1)))
        dpos.append(np.ascontiguousarray(
            np.concatenate(dp, 1).astype(ml_dtypes.bfloat16)))

    binload = deg[arr].sum(axis=0)
    return dict(
        perm=perm, TL=TL, TH=TH, NPC=NPC, NTOT=NTOT,
        idxlow=idxlow, idxhigh=idxhigh, idxer=idxer, dpos=dpos,
        binload=binload,
    )


# ----------------------------------------------------------------------------
# Device program
# ----------------------------------------------------------------------------

def _build_program(NT, TL, TH, split):
    import concourse.bacc as bacc
    import concourse.mybir as mybir
    import concourse.tile as tile

    dt = mybir.dt
    F = dt.float32r      # 4-byte float, fast PE mode
    F32 = dt.float32
    BF = dt.bfloat16
    I16 = dt.int16
    AF = mybir.ActivationFunctionType
    OP = mybir.AluOpType

    NPC = NT * P
    NTOT = NCORES * NPC
    TT = TL + TH
    assert NT % SUP == 0
    NSUP = NT // SUP
    NLO = SUP * TL * P       # low slots per super
    NHI = SUP * TH * P
    CL, CH, CE = NLO // 16, NHI // 16, (NLO + NHI) // 16

    nc = bacc.Bacc("TRN2", target_bir_lowering=False, debug=False,
                   num_devices=NCORES,
                   dynamic_dma_scratch_size=int(os.environ.get("GNN_SCRATCH")
                                                or 16384),
                   num_swdge_queues=int(os.environ.get("GNN_QUEUES") or 1))

    obst_p = nc.dram_tensor("obst", [OBS_D, NPC], F, kind="ExternalInput")
    W1_p = nc.dram_tensor("w1", [OBS_D, HID], F, kind="ExternalInput")
    b1_p = nc.dram_tensor("b1", [HID, 1], F32, kind="ExternalInput")
    W2_p = nc.dram_tensor("w2", [HID, H], F, kind="ExternalInput")
    b2_p = nc.dram_tensor("b2", [H, 1], F32, kind="ExternalInput")
    Wg_p = [nc.dram_tensor(f"wg{i}", [H, H], BF, kind="ExternalInput")
            for i in (1, 2)]
    Wgal_p = [nc.dram_tensor(f"wgal{i}", [H, NH], BF, kind="ExternalInput")
              for i in (1, 2)]
    Wgar_p = [nc.dram_tensor(f"wgar{i}", [H, NH], BF, kind="ExternalInput")
              for i in (1, 2)]
    iota_p = nc.dram_tensor("iota", [P, P], BF, kind="ExternalInput")
    identf_p = nc.dram_tensor("identf", [P, P], F, kind="ExternalInput")
    il_p = nc.dram_tensor("idxlow", [P, NSUP * CL], I16, kind="ExternalInput")
    ih_p = nc.dram_tensor("idxhigh", [P, NSUP * CH], I16, kind="ExternalInput")
    ie_p = nc.dram_tensor("idxer", [P, NSUP * CE], I16, kind="ExternalInput")
    dpos_p = nc.dram_tensor("dposslab", [P, NT * TT], BF, kind="ExternalInput")
    out_p = nc.dram_tensor("out", [NPC, 3 * H], F, kind="ExternalOutput")

    tab_loc = [nc.dram_tensor(f"tab{i}_loc", [NPC, TABW], BF) for i in (1, 2)]
    tab_full = [nc.dram_tensor(f"tab{i}_full", [NTOT, TABW], BF,
                               addr_space="Shared") for i in (1, 2)]
    er_tab = [nc.dram_tensor(f"er{i}_tab", [NPC, ERW], BF) for i in (1, 2)]

    groups = [list(range(NCORES))]

    with tile.TileContext(nc) as tc:
        with (
            tc.tile_pool(name="const", bufs=1) as constp,
            tc.tile_pool(name="obst", bufs=2) as obstp,
            tc.tile_pool(name="enc", bufs=2) as encp,
            tc.tile_pool(name="rows", bufs=3) as rowsp,
            tc.tile_pool(name="idx", bufs=3) as idxp,
            tc.tile_pool(name="gath", bufs=3) as gathp,
            tc.tile_pool(name="small", bufs=3) as smallp,
            tc.tile_pool(name="rhs", bufs=3) as rhsp,
            tc.tile_pool(name="bt", bufs=3) as btp,
            tc.tile_pool(name="pe", bufs=2, space="PSUM") as pep,
            tc.tile_pool(name="pacc", bufs=3, space="PSUM") as paccp,
            tc.tile_pool(name="ptr", bufs=1, space="PSUM") as ptrp,
            tc.tile_pool(name="prod", bufs=2, space="PSUM") as prodp,
        ):
            # ---------------- prologue ----------------
            iota_sb = constp.tile([P, P], BF, tag="iota")
            nc.sync.dma_start(iota_sb[:], iota_p[:, :])
            ident = constp.tile([P, P], F, tag="ident")
            nc.sync.dma_start(ident[:], identf_p[:, :])

            W1_sb = []
            for k in range(2):
                t = constp.tile([P, HID], F, tag=f"w1_{k}")
                nc.sync.dma_start(t[:], W1_p[k * P:(k + 1) * P, :])
                W1_sb.append(t)
            W2_sb = []
            for m in range(4):
                t = constp.tile([P, H], F, tag=f"w2_{m}")
                nc.sync.dma_start(t[:], W2_p[m * P:(m + 1) * P, :])
                W2_sb.append(t)
            b1_sb = []
            for m in range(4):
                t = constp.tile([P, 1], F32, tag=f"b1_{m}")
                nc.sync.dma_start(t[:], b1_p[m * P:(m + 1) * P, :])
                b1_sb.append(t)
            b2_sb = constp.tile([P, 1], F32, tag="b2")
            nc.sync.dma_start(b2_sb[:], b2_p[:, :])
            Wg_sb, Wgal_sb, Wgar_sb = [], [], []
            for i in range(2):
                t = constp.tile([P, H], BF, tag=f"wg_{i}")
                nc.sync.dma_start(t[:], Wg_p[i][:, :])
                Wg_sb.append(t)
                t = constp.tile([P, NH], BF, tag=f"wgal_{i}")
                nc.sync.dma_start(t[:], Wgal_p[i][:, :])
                Wgal_sb.append(t)
                t = constp.tile([P, NH], BF, tag=f"wgar_{i}")
                nc.sync.dma_start(t[:], Wgar_p[i][:, :])
                Wgar_sb.append(t)

            def table_products(zTb_chunk, row0, li):
                pr = prodp.tile([P, H + 2 * NH], F32, tag="pr")
                nc.tensor.matmul(pr[:, 0:H], lhsT=zTb_chunk, rhs=Wg_sb[li][:],
                                 start=True, stop=True)
                nc.tensor.matmul(pr[:, H:H + NH], lhsT=zTb_chunk,
                                 rhs=Wgal_sb[li][:], start=True, stop=True)
                nc.tensor.matmul(pr[:, H + NH:H + 2 * NH], lhsT=zTb_chunk,
                                 rhs=Wgar_sb[li][:], start=True, stop=True)
                tabt = rowsp.tile([P, TAB], BF, tag="tabt")
                nc.vector.tensor_copy(tabt[:], pr[:, 0:TAB])
                nc.sync.dma_start(tab_loc[li][row0:row0 + P, 0:TAB], tabt[:])
                ert = rowsp.tile([P, NH], BF, tag="ert")
                nc.vector.tensor_copy(ert[:], pr[:, H + NH:H + 2 * NH])
                nc.sync.dma_start(er_tab[li][row0:row0 + P, 0:NH], ert[:])

            # ---------------- phase E: encoder ----------------
            for pt in range(NT // 2):
                n0 = pt * 2 * P
                obsT = []
                for k in range(2):
                    t = obstp.tile([P, 2 * P], F, tag="obsT")
                    nc.sync.dma_start(t[:], obst_p[k * P:(k + 1) * P,
                                                   n0:n0 + 2 * P])
                    obsT.append(t)
                hT = []
                for m in range(4):
                    ph = pep.tile([P, 2 * P], F32, tag="pe")
                    for k in range(2):
                        nc.tensor.matmul(
                            ph[:], lhsT=W1_sb[k][:, m * P:(m + 1) * P],
                            rhs=obsT[k][:], start=(k == 0), stop=(k == 1))
                    h = encp.tile([P, 2 * P], F, tag=f"h{m}")
                    nc.vector.tensor_scalar(
                        out=h[:], in0=ph[:], scalar1=b1_sb[m][:, 0:1],
                        scalar2=0.0, op0=OP.add, op1=OP.max)
                    hT.append(h)
                pz = pep.tile([P, 2 * P], F32, tag="pe")
                for m in range(4):
                    nc.tensor.matmul(pz[:], lhsT=W2_sb[m][:], rhs=hT[m][:],
                                     start=(m == 0), stop=(m == 3))
                z1T = encp.tile([P, 2 * P], F, tag="z1T")
                nc.vector.tensor_scalar(
                    out=z1T[:], in0=pz[:], scalar1=b2_sb[:, 0:1],
                    scalar2=0.0, op0=OP.add, op1=OP.max)
                z1Tb = encp.tile([P, 2 * P], BF, tag="z1Tb")
                nc.vector.tensor_copy(z1Tb[:], z1T[:])
                for k in range(2):
                    ptr = ptrp.tile([P, P], F, tag="ptr")
                    nc.tensor.transpose(ptr[:], z1T[:, k * P:(k + 1) * P],
                                        ident[:])
                    zrow = rowsp.tile([P, P], F, tag="zrows")
                    nc.vector.tensor_copy(zrow[:], ptr[:])
                    nc.sync.dma_start(
                        out_p[n0 + k * P:n0 + (k + 1) * P, 0:H], zrow[:])
                    table_products(z1Tb[:, k * P:(k + 1) * P], n0 + k * P, 0)

            nc.gpsimd.collective_compute(
                "AllGather", OP.bypass, replica_groups=groups,
                ins=[tab_loc[0][:, :]], outs=[tab_full[0][:, :]])

            # ---------------- edge pass ----------------
            def epilogue(D, acc, out_col, build_next):
                zp = smallp.tile([P, NH], F32, tag="zp")
                nc.vector.tensor_scalar_add(zp[:], acc[:, H:TAB], 1e-9)
                zrec = smallp.tile([P, NH], F32, tag="zrec")
                nc.vector.reciprocal(zrec[:], zp[:])
                zo = rowsp.tile([P, H], F, tag="zo")
                nc.vector.scalar_tensor_tensor(
                    out=zo[:].rearrange("p (h d) -> p h d", h=NH),
                    in0=acc[:, 0:H].rearrange("p (h d) -> p h d", h=NH),
                    scalar=0.0,
                    in1=zrec[:].unsqueeze(2).to_broadcast([P, NH, HD]),
                    op0=OP.max, op1=OP.mult)
                nc.sync.dma_start(
                    out_p[D * P:(D + 1) * P, out_col:out_col + H], zo[:])
                if build_next:
                    pzt = ptrp.tile([P, P], F, tag="ptr")
                    nc.tensor.transpose(pzt[:], zo[:], ident[:])
                    zTb = rowsp.tile([P, P], BF, tag="zTb")
                    nc.vector.tensor_copy(zTb[:], pzt[:])
                    table_products(zTb[:], D * P, 1)

            def edge_block(g, erc, er_off, dpos_sb, dp_off, accs,
                           S, TB, out_col, build_next, is_high):
                """One low/high block of a super: batched attn + per-tile mm."""
                n = SUP * TB
                g3 = g[:].rearrange("p (c e) -> p c e", e=TABW)
                e_t = smallp.tile([P, n * NH], F32,
                                  tag="e_th" if is_high else "e_tl")
                nc.vector.tensor_add(
                    e_t[:].rearrange("p (c e) -> p c e", e=NH),
                    g3[:, :, H:TAB],
                    erc[:].rearrange("p (c e) -> p c e", e=NH)
                       [:, er_off:er_off + n, :])
                ex1 = smallp.tile([P, n * NH], F32,
                                  tag="ex1h" if is_high else "ex1l")
                nc.scalar.activation(ex1[:], e_t[:], AF.Exp)
                ex2 = smallp.tile([P, n * NH], F32,
                                  tag="ex2h" if is_high else "ex2l")
                nc.scalar.activation(ex2[:], e_t[:], AF.Exp, scale=0.2)
                rhs = rhsp.tile([P, n * TAB], BF,
                                tag="rhsh" if is_high else "rhsl")
                r3 = rhs[:].rearrange("p (c e) -> p c e", e=TAB)
                nc.vector.tensor_max(
                    r3[:, :, H:TAB],
                    ex1[:].rearrange("p (c e) -> p c e", e=NH),
                    ex2[:].rearrange("p (c e) -> p c e", e=NH))
                bt = btp.tile([P, n * P], BF,
                              tag="bth" if is_high else "btl")
                nc.vector.tensor_tensor(
                    out=bt[:].rearrange("p (c e) -> p c e", e=P),
                    in0=dpos_sb[:, dp_off:dp_off + n].unsqueeze(2)
                        .to_broadcast([P, n, P]),
                    in1=iota_sb[:].unsqueeze(1).to_broadcast([P, n, P]),
                    op=OP.is_equal)
                nc.vector.tensor_tensor(
                    out=r3[:, :, 0:H].rearrange("p c (h d) -> p c h d", h=NH),
                    in0=g3[:, :, 0:H].rearrange("p c (h d) -> p c h d", h=NH),
                    in1=r3[:, :, H:TAB].unsqueeze(3)
                        .to_broadcast([P, n, NH, HD]),
                    op=OP.mult)
                for c in range(n):
                    b = c // TB
                    t = c % TB
                    D = S * SUP + b
                    if not is_high and t == 0:
                        acc_new = paccp.tile([P, TAB], F32, tag="acc")
                        accs[b] = acc_new
                    nc.tensor.matmul(
                        accs[b][:], lhsT=bt[:, c * P:(c + 1) * P],
                        rhs=r3[:, c, :],
                        start=(not is_high and t == 0),
                        stop=(is_high and t == TB - 1))
                    if is_high and t == TB - 1:
                        epilogue(D, accs[b], out_col, build_next)

            def edge_pass(li, out_col, build_next):
                tabf = tab_full[li]
                ert_d = er_tab[li]
                accs = [None] * SUP
                for S in range(NSUP):
                    ilow = idxp.tile([P, CL], I16, tag="ilow")
                    nc.sync.dma_start(ilow[:], il_p[:, S * CL:(S + 1) * CL])
                    ihigh = idxp.tile([P, CH], I16, tag="ihigh")
                    nc.sync.dma_start(ihigh[:], ih_p[:, S * CH:(S + 1) * CH])
                    ier = idxp.tile([P, CE], I16, tag="ier")
                    nc.sync.dma_start(ier[:], ie_p[:, S * CE:(S + 1) * CE])
                    dpos_sb = idxp.tile([P, SUP * TT], BF, tag="dpos")
                    nc.sync.dma_start(
                        dpos_sb[:],
                        dpos_p[:, S * SUP * TT:(S + 1) * SUP * TT])

                    glow = gathp.tile([P, SUP * TL * TABW], BF, tag="glow")
                    nc.gpsimd.dma_gather(
                        out_ap=glow[:].rearrange("p (c e) -> p c e", e=TABW),
                        in_ap=tabf[:, :], idxs_ap=ilow[:],
                        num_idxs=NLO, num_idxs_reg=NLO, elem_size=TABW,
                        single_packet=False, queue_num=0)
                    ghigh = gathp.tile([P, SUP * TH * TABW], BF, tag="ghigh")
                    nc.gpsimd.dma_gather(
                        out_ap=ghigh[:].rearrange("p (c e) -> p c e", e=TABW),
                        in_ap=tabf[split:NTOT, :], idxs_ap=ihigh[:],
                        num_idxs=NHI, num_idxs_reg=NHI, elem_size=TABW,
                        single_packet=False,
                        queue_num=1 % int(os.environ.get('GNN_QUEUES') or 1))
                    erg = gathp.tile([P, SUP * TT * ERW], BF, tag="erg")
                    nc.gpsimd.dma_gather(
                        out_ap=erg[:].rearrange("p (c e) -> p c e", e=ERW),
                        in_ap=ert_d[:, :], idxs_ap=ier[:],
                        num_idxs=NLO + NHI, num_idxs_reg=NLO + NHI,
                        elem_size=ERW, single_packet=False,
                        queue_num=2 % int(os.environ.get('GNN_QUEUES') or 1))
                    # compact er columns (0:8 of each 128-wide row)
                    erc = smallp.tile([P, SUP * TT * NH], BF, tag="erc")
                    nc.vector.tensor_copy(
                        erc[:].rearrange("p (c e) -> p c e", e=NH),
                        erg[:].rearrange("p (c e) -> p c e", e=ERW)[:, :, 0:NH])
                    edge_block(glow, erc, 0, dpos_sb, 0, accs,
                               S, TL, out_col, build_next, False)
                    edge_block(ghigh, erc, SUP * TL, dpos_sb,
                               SUP * TL, accs, S, TH, out_col, build_next,
                               True)

            edge_pass(0, H, True)

            nc.gpsimd.collective_compute(
                "AllGather", OP.bypass, replica_groups=groups,
                ins=[tab_loc[1][:, :]], outs=[tab_full[1][:, :]])

            edge_pass(1, 2 * H, False)

    nc.compile()
    return nc


# ----------------------------------------------------------------------------
# Driver
# ----------------------------------------------------------------------------

def _make_blockdiag(a):
    bd = np.zeros((H, NH), np.float32)
    for h in range(NH):
        bd[h * HD:(h + 1) * HD, h] = a[h]
    return bd


def run_gnn(inputs, n_tiles_per_core=52, trace=False):
    import ml_dtypes
    bf16 = ml_dtypes.bfloat16

    t_start = time.time()
    obs = np.asarray(inputs["obs"], np.float32)
    src = np.asarray(inputs["src"], np.int64)
    dst = np.asarray(inputs["dst"], np.int64)
    N = obs.shape[0]

    NTOT_ = NCORES * n_tiles_per_core * P
    split = min(SPLIT, NTOT_ // 2)
    prep = _host_prepare(src, dst, n_tiles_per_core, split)
    NT = n_tiles_per_core
    TL, TH, NPC, NTOT = prep["TL"], prep["TH"], prep["NPC"], prep["NTOT"]
    perm = prep["perm"]

    al1bd = _make_blockdiag(np.asarray(inputs["al1"], np.float32))
    ar1bd = _make_blockdiag(np.asarray(inputs["ar1"], np.float32))
    al2bd = _make_blockdiag(np.asarray(inputs["al2"], np.float32))
    ar2bd = _make_blockdiag(np.asarray(inputs["ar2"], np.float32))
    Wg1 = np.asarray(inputs["Wg1"], np.float32)
    Wg2 = np.asarray(inputs["Wg2"], np.float32)
    shared = {
        "w1": np.asarray(inputs["W1"], np.float32),
        "b1": np.asarray(inputs["b1"], np.float32).reshape(HID, 1),
        "w2": np.asarray(inputs["W2"], np.float32),
        "b2": np.asarray(inputs["b2"], np.float32).reshape(H, 1),
        "wg1": Wg1.astype(bf16), "wg2": Wg2.astype(bf16),
        "wgal1": (Wg1 @ al1bd).astype(bf16),
        "wgar1": (Wg1 @ ar1bd).astype(bf16),
        "wgal2": (Wg2 @ al2bd).astype(bf16),
        "wgar2": (Wg2 @ ar2bd).astype(bf16),
        "iota": np.tile(np.arange(P, dtype=np.float32)[None, :],
                        (P, 1)).astype(bf16),
        "identf": np.eye(P, dtype=np.float32),
    }

    obs_pad = np.zeros((NTOT, OBS_D), np.float32)
    obs_pad[:N] = obs
    obs_perm = obs_pad[perm]

    in_maps = []
    for c in range(NCORES):
        m = dict(shared)
        m["obst"] = np.ascontiguousarray(obs_perm[c * NPC:(c + 1) * NPC].T)
        m["idxlow"] = prep["idxlow"][c]
        m["idxhigh"] = prep["idxhigh"][c]
        m["idxer"] = prep["idxer"][c]
        m["dposslab"] = prep["dpos"][c]
        in_maps.append(m)

    t_prep = time.time()
    nc = _build_program(NT, TL, TH, split)
    t_build = time.time()

    from concourse.bass_utils import run_bass_kernel_spmd
    if trace:
        _ensure_ntff_hook()
    res = run_bass_kernel_spmd(nc, in_maps, core_ids=list(range(NCORES)),
                               trace=trace)
    t_run = time.time()

    full = np.concatenate([res.results[c]["out"] for c in range(NCORES)],
                          axis=0)
    out = np.empty((N, 3 * H), np.float32)
    keep = perm < N
    out[perm[keep]] = full[keep]

    LAST_INFO.clear()
    LAST_INFO.update(dict(
        exec_time_ns=res.exec_time_ns, TL=TL, TH=TH,
        binload_max=int(prep["binload"].max()),
        t_prep=t_prep - t_start, t_build=t_build - t_prep,
        t_run=t_run - t_build,
        profile_json=getattr(res, "profile_json", None),
    ))
    return out


def kernel(**inputs):
    return run_gnn(inputs, n_tiles_per_core=52,
                   trace=bool(os.environ.get("GNN_TRACE")))


# revision 16
# speedup vs baseline: 1.9295x; 1.6304x over previous
"""Distributed multi-head GAT (encoder + 2 GAT layers) on 8 TRN2 NeuronCores.

Strategy (graph/data parallel, per the dst-ownership sharding):
  * Nodes are permuted and dealt into 8*NT bins of 128 nodes so that every
    bin (= one PSUM dst-tile) has a near-equal number of incoming edges and
    every core has a near-equal total.  Edges live with the core that owns
    their dst node.
  * Each core encodes its own node shard (obs -> z1), builds a per-node
    bf16 table row [feat(128) | el(8) | pad] (512B stride for dma_gather),
    and the 8 shards are AllGathered into a full table so any core can
    gather src rows (the "halo" of a random graph is the full table).
  * Edge pass: each bin's edges are split into low-src (table row < 32768)
    and high-src tile groups so the batched int16 `dma_gather` can address
    the table; one gather per super-tile per group + one local er gather.
    Batched DVE ops compute ex = exp(leaky_relu(el+er)) (exactly
    max(exp(x), exp(0.2x))) and scale messages; per 128-edge tile a one-hot
    matmul (Bt[e,d] = [dstpos[e] == d]) reduces into the bin's PSUM
    accumulator.  The softmax max-subtraction is skipped: inputs are O(0.1)
    so exp is safe, and the result matches up to the 1e-9 epsilon scaling.
  * Epilogue per bin: out = relu(acc_feat) * 1/(acc_z + 1e-9) per head;
    also builds the next layer's table row + er entries.
"""

import os
import sys
import time

import numpy as np

for _p in ("/opt/trn_rl_repo", "/root/.axon_site/_ro/trn_rl_repo"):
    if os.path.isdir(_p) and _p not in sys.path:
        sys.path.insert(0, _p)

P = 128
NCORES = 8
OBS_D = 256
HID = 512
H = 128          # h_dim
NH = 8           # heads
HD = 16          # head dim
TABW = 256       # table row width (bf16) -> 512B stride; cols 0:136 used
TAB = H + NH     # useful table row: feat | el
ERW = 128        # er table row width (bf16) -> 256B stride; cols 0:8 used
SUP = 2          # bins per gather super-tile
SPLIT = 32768    # low/high table split for int16 gather indices
PAD_SENTINEL = 200.0

LAST_INFO = {}


def _ensure_ntff_hook():
    """Register the axon NTFF profile hook if the image's antenv lacks it."""
    try:
        import types

        import antenv
        try:
            from antenv import axon_hooks  # noqa: F401
            return
        except ImportError:
            pass
        m = types.ModuleType("antenv.axon_hooks")
        _h = [None]
        m.set_axon_ntff_profile_hook = lambda hook: _h.__setitem__(0, hook)
        m.get_axon_ntff_profile_hook = lambda: _h[0]
        sys.modules["antenv.axon_hooks"] = m
        antenv.axon_hooks = m
        from trn_agent_boot.trn_boot import _ntff_profile_via_ctypes
        m.set_axon_ntff_profile_hook(
            _ntff_profile_via_ctypes("/opt/axon/libaxon_pjrt.so"))
    except Exception as e:  # profiling is best-effort
        print(f"ntff hook setup failed: {e}")


# ----------------------------------------------------------------------------
# Host-side preprocessing
# ----------------------------------------------------------------------------

def _wrap16(a):
    """[n] -> [128, n/16] int16 in the dma_gather wrapped layout:
    index j lives at partition j%16, col j//16, replicated to all 8 groups."""
    n = a.shape[0]
    w = a.reshape(n // 16, 16).T.astype(np.int16)    # [16, n/16]
    return np.ascontiguousarray(np.tile(w, (8, 1)))


def _host_prepare(src, dst, n_tiles_per_core, split):
    """Balance bins, split edges by src table half, build gather slabs."""
    import ml_dtypes

    NT = n_tiles_per_core
    NPC = NT * P
    NTOT = NCORES * NPC
    NBINS = NCORES * NT
    E = src.shape[0]

    deg = np.bincount(dst, minlength=NTOT).astype(np.int64)
    order = np.argsort(-deg, kind="stable")
    arr = order.reshape(P, NBINS).copy()
    arr[1::2] = arr[1::2, ::-1]
    perm = arr.T.reshape(-1)                       # g -> orig node
    pos = np.empty(NTOT, np.int64)
    pos[perm] = np.arange(NTOT)                    # orig node -> g

    srcg = pos[src]
    dstg = pos[dst]
    binid = dstg // P
    low = srcg < split

    nlo = np.bincount(binid[low], minlength=NBINS)
    nhi = np.bincount(binid[~low], minlength=NBINS)
    TL = max(1, int(np.ceil(nlo.max() / P)))
    TH = max(1, int(np.ceil(nhi.max() / P)))
    TT = TL + TH

    gkey = binid * 2 + (~low).astype(np.int64)
    eorder = np.argsort(gkey, kind="stable")
    counts = np.bincount(gkey, minlength=2 * NBINS)
    starts = np.concatenate([[0], np.cumsum(counts)[:-1]])
    rank = np.arange(E) - starts[gkey[eorder]]
    ghigh = gkey[eorder] % 2
    slot = (gkey[eorder] // 2) * (TT * P) + ghigh * (TL * P) + rank

    ES = NBINS * TT * P
    sg = np.zeros(ES, np.int64)
    isreal = np.zeros(ES, bool)
    dlocal = np.zeros(ES, np.int64)
    dposf = np.full(ES, PAD_SENTINEL, np.float32)
    sg[slot] = srcg[eorder]
    isreal[slot] = True
    dlocal[slot] = dstg[eorder] % NPC
    dposf[slot] = (dstg[eorder] % P).astype(np.float32)

    # high-section pads must index within the high table half
    sg[~isreal & (np.arange(ES) % (TT * P) >= TL * P)] = split

    sg = sg.reshape(NCORES, NT, TT * P)
    dlocal = dlocal.reshape(NCORES, NT, TT * P)
    dposf = dposf.reshape(NCORES, NT, TT * P).astype(ml_dtypes.bfloat16)

    NSUP = NT // SUP
    idxlow, idxhigh, dpos, dprow = [], [], [], []
    for c in range(NCORES):
        il, ih, dp, dr = [], [], [], []
        for S in range(NSUP):
            b0 = S * SUP
            lo_slots = sg[c, b0:b0 + SUP, :TL * P].reshape(-1)
            hi_slots = sg[c, b0:b0 + SUP, TL * P:].reshape(-1) - split
            il.append(_wrap16(lo_slots))
            ih.append(_wrap16(hi_slots))
            dpl = dposf[c, b0:b0 + SUP, :TL * P].reshape(SUP * TL, P)
            dph = dposf[c, b0:b0 + SUP, TL * P:].reshape(SUP * TH, P)
            both = np.concatenate([dpl, dph], 0)         # [SUP*TT, 128]
            dp.append(both.T)                            # [128, SUP*TT]
            dr.append(both.reshape(-1))                  # slot-order flat
        idxlow.append(np.ascontiguousarray(np.concatenate(il, 1)))
        idxhigh.append(np.ascontiguousarray(np.concatenate(ih, 1)))
        dpos.append(np.ascontiguousarray(
            np.concatenate(dp, 1).astype(ml_dtypes.bfloat16)))
        dprow.append(np.ascontiguousarray(
            np.concatenate(dr)[None, :].astype(ml_dtypes.bfloat16)))

    binload = deg[arr].sum(axis=0)
    return dict(
        perm=perm, TL=TL, TH=TH, NPC=NPC, NTOT=NTOT,
        idxlow=idxlow, idxhigh=idxhigh, dpos=dpos, dprow=dprow,
        binload=binload,
    )


# ----------------------------------------------------------------------------
# Device program
# ----------------------------------------------------------------------------

def _build_program(NT, TL, TH, split):
    import concourse.bacc as bacc
    import concourse.mybir as mybir
    import concourse.tile as tile

    dt = mybir.dt
    F = dt.float32r      # 4-byte float, fast PE mode
    F32 = dt.float32
    BF = dt.bfloat16
    I16 = dt.int16
    AF = mybir.ActivationFunctionType
    OP = mybir.AluOpType

    NPC = NT * P
    NTOT = NCORES * NPC
    TT = TL + TH
    assert NT % SUP == 0
    NSUP = NT // SUP
    NLO = SUP * TL * P       # low slots per super
    NHI = SUP * TH * P
    CL, CH, CE = NLO // 16, NHI // 16, (NLO + NHI) // 16

    nc = bacc.Bacc("TRN2", target_bir_lowering=False, debug=False,
                   num_devices=NCORES,
                   dynamic_dma_scratch_size=int(os.environ.get("GNN_SCRATCH")
                                                or 16384),
                   num_swdge_queues=int(os.environ.get("GNN_QUEUES") or 1))

    obst_p = nc.dram_tensor("obst", [OBS_D, NPC], F, kind="ExternalInput")
    W1_p = nc.dram_tensor("w1", [OBS_D, HID], F, kind="ExternalInput")
    b1_p = nc.dram_tensor("b1", [HID, 1], F32, kind="ExternalInput")
    W2_p = nc.dram_tensor("w2", [HID, H], F, kind="ExternalInput")
    b2_p = nc.dram_tensor("b2", [H, 1], F32, kind="ExternalInput")
    Wg_p = [nc.dram_tensor(f"wg{i}", [H, H], BF, kind="ExternalInput")
            for i in (1, 2)]
    Wgal_p = [nc.dram_tensor(f"wgal{i}", [H, NH], BF, kind="ExternalInput")
              for i in (1, 2)]
    Wgar_p = [nc.dram_tensor(f"wgar{i}", [H, NH], BF, kind="ExternalInput")
              for i in (1, 2)]
    iota_p = nc.dram_tensor("iota", [P, P], BF, kind="ExternalInput")
    identf_p = nc.dram_tensor("identf", [P, P], F, kind="ExternalInput")
    il_p = nc.dram_tensor("idxlow", [P, NSUP * CL], I16, kind="ExternalInput")
    ih_p = nc.dram_tensor("idxhigh", [P, NSUP * CH], I16, kind="ExternalInput")
    dprow_p = nc.dram_tensor("dprow", [1, NT * TT * P], BF,
                             kind="ExternalInput")
    iotac_p = nc.dram_tensor("iotac", [P, 1], BF, kind="ExternalInput")
    dpos_p = nc.dram_tensor("dposslab", [P, NT * TT], BF, kind="ExternalInput")
    out_p = nc.dram_tensor("out", [NPC, 3 * H], F, kind="ExternalOutput")

    tab_loc = [nc.dram_tensor(f"tab{i}_loc", [NPC, TABW], BF) for i in (1, 2)]
    tab_full = [nc.dram_tensor(f"tab{i}_full", [NTOT, TABW], BF,
                               addr_space="Shared") for i in (1, 2)]

    groups = [list(range(NCORES))]

    with tile.TileContext(nc) as tc:
        with (
            tc.tile_pool(name="const", bufs=1) as constp,
            tc.tile_pool(name="obst", bufs=2) as obstp,
            tc.tile_pool(name="enc", bufs=2) as encp,
            tc.tile_pool(name="rows", bufs=3) as rowsp,
            tc.tile_pool(name="idx", bufs=3) as idxp,
            tc.tile_pool(name="gath", bufs=3) as gathp,
            tc.tile_pool(name="small", bufs=3) as smallp,
            tc.tile_pool(name="rhs", bufs=3) as rhsp,
            tc.tile_pool(name="bt", bufs=3) as btp,
            tc.tile_pool(name="pe", bufs=2, space="PSUM") as pep,
            tc.tile_pool(name="pacc", bufs=2, space="PSUM") as paccp,
            tc.tile_pool(name="ptr", bufs=1, space="PSUM") as ptrp,
            tc.tile_pool(name="prod", bufs=1, space="PSUM") as prodp,
            tc.tile_pool(name="pers", bufs=2, space="PSUM") as persp,
        ):
            # ---------------- prologue ----------------
            iota_sb = constp.tile([P, P], BF, tag="iota")
            nc.sync.dma_start(iota_sb[:], iota_p[:, :])
            ident = constp.tile([P, P], F, tag="ident")
            nc.sync.dma_start(ident[:], identf_p[:, :])
            iotac_sb = constp.tile([P, 1], BF, tag="iotac")
            nc.sync.dma_start(iotac_sb[:], iotac_p[:, :])
            er_sb0 = constp.tile([P, NT * NH], BF, tag="er_sb0")
            er_sb1 = constp.tile([P, NT * NH], BF, tag="er_sb1")
            er_sb = [er_sb0, er_sb1]

            W1_sb = []
            for k in range(2):
                t = constp.tile([P, HID], F, tag=f"w1_{k}")
                nc.sync.dma_start(t[:], W1_p[k * P:(k + 1) * P, :])
                W1_sb.append(t)
            W2_sb = []
            for m in range(4):
                t = constp.tile([P, H], F, tag=f"w2_{m}")
                nc.sync.dma_start(t[:], W2_p[m * P:(m + 1) * P, :])
                W2_sb.append(t)
            b1_sb = []
            for m in range(4):
                t = constp.tile([P, 1], F32, tag=f"b1_{m}")
                nc.sync.dma_start(t[:], b1_p[m * P:(m + 1) * P, :])
                b1_sb.append(t)
            b2_sb = constp.tile([P, 1], F32, tag="b2")
            nc.sync.dma_start(b2_sb[:], b2_p[:, :])
            Wg_sb, Wgal_sb, Wgar_sb = [], [], []
            for i in range(2):
                t = constp.tile([P, H], BF, tag=f"wg_{i}")
                nc.sync.dma_start(t[:], Wg_p[i][:, :])
                Wg_sb.append(t)
                t = constp.tile([P, NH], BF, tag=f"wgal_{i}")
                nc.sync.dma_start(t[:], Wgal_p[i][:, :])
                Wgal_sb.append(t)
                t = constp.tile([P, NH], BF, tag=f"wgar_{i}")
                nc.sync.dma_start(t[:], Wgar_p[i][:, :])
                Wgar_sb.append(t)

            def table_products(zTb_chunk, row0, li):
                pr = prodp.tile([P, H + 2 * NH], F32, tag="pr")
                nc.tensor.matmul(pr[:, 0:H], lhsT=zTb_chunk, rhs=Wg_sb[li][:],
                                 start=True, stop=True)
                nc.tensor.matmul(pr[:, H:H + NH], lhsT=zTb_chunk,
                                 rhs=Wgal_sb[li][:], start=True, stop=True)
                nc.tensor.matmul(pr[:, H + NH:H + 2 * NH], lhsT=zTb_chunk,
                                 rhs=Wgar_sb[li][:], start=True, stop=True)
                tabt = rowsp.tile([P, TAB], BF, tag="tabt")
                nc.vector.tensor_copy(tabt[:], pr[:, 0:TAB])
                nc.sync.dma_start(tab_loc[li][row0:row0 + P, 0:TAB], tabt[:])
                D = row0 // P
                nc.vector.tensor_copy(er_sb[li][:, D * NH:(D + 1) * NH],
                                      pr[:, H + NH:H + 2 * NH])

            # ---------------- phase E: encoder ----------------
            for pt in range(NT // 2):
                n0 = pt * 2 * P
                obsT = []
                for k in range(2):
                    t = obstp.tile([P, 2 * P], F, tag="obsT")
                    nc.sync.dma_start(t[:], obst_p[k * P:(k + 1) * P,
                                                   n0:n0 + 2 * P])
                    obsT.append(t)
                hT = []
                for m in range(4):
                    ph = pep.tile([P, 2 * P], F32, tag="pe")
                    for k in range(2):
                        nc.tensor.matmul(
                            ph[:], lhsT=W1_sb[k][:, m * P:(m + 1) * P],
                            rhs=obsT[k][:], start=(k == 0), stop=(k == 1))
                    h = encp.tile([P, 2 * P], F, tag=f"h{m}")
                    nc.vector.tensor_scalar(
                        out=h[:], in0=ph[:], scalar1=b1_sb[m][:, 0:1],
                        scalar2=0.0, op0=OP.add, op1=OP.max)
                    hT.append(h)
                pz = pep.tile([P, 2 * P], F32, tag="pe")
                for m in range(4):
                    nc.tensor.matmul(pz[:], lhsT=W2_sb[m][:], rhs=hT[m][:],
                                     start=(m == 0), stop=(m == 3))
                z1T = encp.tile([P, 2 * P], F, tag="z1T")
                nc.vector.tensor_scalar(
                    out=z1T[:], in0=pz[:], scalar1=b2_sb[:, 0:1],
                    scalar2=0.0, op0=OP.add, op1=OP.max)
                z1Tb = encp.tile([P, 2 * P], BF, tag="z1Tb")
                nc.vector.tensor_copy(z1Tb[:], z1T[:])
                for k in range(2):
                    ptr = ptrp.tile([P, P], F, tag="ptr")
                    nc.tensor.transpose(ptr[:], z1T[:, k * P:(k + 1) * P],
                                        ident[:])
                    zrow = rowsp.tile([P, P], F, tag="zrows")
                    nc.vector.tensor_copy(zrow[:], ptr[:])
                    nc.sync.dma_start(
                        out_p[n0 + k * P:n0 + (k + 1) * P, 0:H], zrow[:])
                    table_products(z1Tb[:, k * P:(k + 1) * P], n0 + k * P, 0)

            nc.gpsimd.collective_compute(
                "AllGather", OP.bypass, replica_groups=groups,
                ins=[tab_loc[0][:, :]], outs=[tab_full[0][:, :]])

            # ---------------- edge pass ----------------
            def epilogue(D, acc, out_col, build_next):
                zp = smallp.tile([P, NH], F32, tag="zp")
                nc.vector.tensor_scalar_add(zp[:], acc[:, H:TAB], 1e-9)
                zrec = smallp.tile([P, NH], F32, tag="zrec")
                nc.vector.reciprocal(zrec[:], zp[:])
                zo = rowsp.tile([P, H], F, tag="zo")
                nc.vector.scalar_tensor_tensor(
                    out=zo[:].rearrange("p (h d) -> p h d", h=NH),
                    in0=acc[:, 0:H].rearrange("p (h d) -> p h d", h=NH),
                    scalar=0.0,
                    in1=zrec[:].unsqueeze(2).to_broadcast([P, NH, HD]),
                    op0=OP.max, op1=OP.mult)
                nc.sync.dma_start(
                    out_p[D * P:(D + 1) * P, out_col:out_col + H], zo[:])
                if build_next:
                    pzt = ptrp.tile([P, P], F, tag="ptr")
                    nc.tensor.transpose(pzt[:], zo[:], ident[:])
                    zTb = rowsp.tile([P, P], BF, tag="zTb")
                    nc.vector.tensor_copy(zTb[:], pzt[:])
                    table_products(zTb[:], D * P, 1)

            def edge_block(g, er_ps, er_off, dpos_sb, dp_off, accs,
                           S, TB, out_col, build_next, is_high):
                """One low/high block of a super: batched attn + per-tile mm."""
                n = SUP * TB
                g3 = g[:].rearrange("p (c e) -> p c e", e=TABW)
                e_t = smallp.tile([P, n * NH], F32,
                                  tag="e_th" if is_high else "e_tl")
                nc.vector.tensor_add(
                    e_t[:].rearrange("p (c e) -> p c e", e=NH),
                    g3[:, :, H:TAB],
                    er_ps[:, er_off * NH:(er_off + n) * NH]
                        .rearrange("p (c e) -> p c e", e=NH))
                ex1 = smallp.tile([P, n * NH], F32,
                                  tag="ex1h" if is_high else "ex1l")
                nc.scalar.activation(ex1[:], e_t[:], AF.Exp)
                ex2 = smallp.tile([P, n * NH], F32,
                                  tag="ex2h" if is_high else "ex2l")
                nc.scalar.activation(ex2[:], e_t[:], AF.Exp, scale=0.2)
                rhs = rhsp.tile([P, n * TAB], BF,
                                tag="rhsh" if is_high else "rhsl")
                r3 = rhs[:].rearrange("p (c e) -> p c e", e=TAB)
                nc.vector.tensor_max(
                    r3[:, :, H:TAB],
                    ex1[:].rearrange("p (c e) -> p c e", e=NH),
                    ex2[:].rearrange("p (c e) -> p c e", e=NH))
                bt = btp.tile([P, n * P], BF,
                              tag="bth" if is_high else "btl")
                nc.vector.tensor_tensor(
                    out=bt[:].rearrange("p (c e) -> p c e", e=P),
                    in0=dpos_sb[:, dp_off:dp_off + n].unsqueeze(2)
                        .to_broadcast([P, n, P]),
                    in1=iota_sb[:].unsqueeze(1).to_broadcast([P, n, P]),
                    op=OP.is_equal)
                nc.vector.tensor_tensor(
                    out=r3[:, :, 0:H].rearrange("p c (h d) -> p c h d", h=NH),
                    in0=g3[:, :, 0:H].rearrange("p c (h d) -> p c h d", h=NH),
                    in1=r3[:, :, H:TAB].unsqueeze(3)
                        .to_broadcast([P, n, NH, HD]),
                    op=OP.mult)
                for c in range(n):
                    b = c // TB
                    t = c % TB
                    D = S * SUP + b
                    if not is_high and t == 0:
                        acc_new = paccp.tile([P, TAB], F32, tag="acc")
                        accs[b] = acc_new
                    nc.tensor.matmul(
                        accs[b][:], lhsT=bt[:, c * P:(c + 1) * P],
                        rhs=r3[:, c, :],
                        start=(not is_high and t == 0),
                        stop=(is_high and t == TB - 1))
                    if is_high and t == TB - 1:
                        epilogue(D, accs[b], out_col, build_next)

            def edge_pass(li, out_col, build_next):
                tabf = tab_full[li]
                accs = [None] * SUP
                for S in range(NSUP):
                    ilow = idxp.tile([P, CL], I16, tag="ilow")
                    nc.sync.dma_start(ilow[:], il_p[:, S * CL:(S + 1) * CL])
                    ihigh = idxp.tile([P, CH], I16, tag="ihigh")
                    nc.sync.dma_start(ihigh[:], ih_p[:, S * CH:(S + 1) * CH])
                    dpos_sb = idxp.tile([P, SUP * TT], BF, tag="dpos")
                    nc.sync.dma_start(
                        dpos_sb[:],
                        dpos_p[:, S * SUP * TT:(S + 1) * SUP * TT])
                    nst = SUP * TT
                    dprow = btp.tile([P, nst * P], BF, tag="dprow")
                    nc.sync.dma_start(
                        dprow[:],
                        dprow_p[0:1, S * nst * P:(S + 1) * nst * P]
                        .to_broadcast([P, nst * P]))
                    ball = btp.tile([P, nst * P], BF, tag="ball")
                    nc.vector.tensor_tensor(
                        out=ball[:].rearrange("p (c e) -> p c e", e=P),
                        in0=iotac_sb[:, 0:1].unsqueeze(2)
                            .to_broadcast([P, nst, P]),
                        in1=dprow[:].rearrange("p (c e) -> p c e", e=P),
                        op=OP.is_equal)
                    er_ps = persp.tile([P, nst * NH], F32, tag="er_ps")
                    for c in range(nst):
                        b = (c // TL) if c < SUP * TL else ((c - SUP * TL)
                                                            // TH)
                        D = S * SUP + b
                        nc.tensor.matmul(
                            er_ps[:, c * NH:(c + 1) * NH],
                            lhsT=ball[:, c * P:(c + 1) * P],
                            rhs=er_sb[li][:, D * NH:(D + 1) * NH],
                            start=True, stop=True)

                    glow = gathp.tile([P, SUP * TL * TABW], BF, tag="glow")
                    # fallthrough
                    nc.gpsimd.dma_gather(
                        out_ap=glow[:].rearrange("p (c e) -> p c e", e=TABW),
                        in_ap=tabf[:, :], idxs_ap=ilow[:],
                        num_idxs=NLO, num_idxs_reg=NLO, elem_size=TABW,
                        single_packet=False, queue_num=0)
                    ghigh = gathp.tile([P, SUP * TH * TABW], BF, tag="ghigh")
                    nc.gpsimd.dma_gather(
                        out_ap=ghigh[:].rearrange("p (c e) -> p c e", e=TABW),
                        in_ap=tabf[split:NTOT, :], idxs_ap=ihigh[:],
                        num_idxs=NHI, num_idxs_reg=NHI, elem_size=TABW,
                        single_packet=False,
                        queue_num=1 % int(os.environ.get('GNN_QUEUES') or 1))
                    edge_block(glow, er_ps, 0, dpos_sb, 0, accs,
                               S, TL, out_col, build_next, False)
                    edge_block(ghigh, er_ps, SUP * TL, dpos_sb,
                               SUP * TL, accs, S, TH, out_col, build_next,
                               True)

            edge_pass(0, H, True)

            nc.gpsimd.collective_compute(
                "AllGather", OP.bypass, replica_groups=groups,
                ins=[tab_loc[1][:, :]], outs=[tab_full[1][:, :]])

            edge_pass(1, 2 * H, False)

    nc.compile()
    return nc


# ----------------------------------------------------------------------------
# Driver
# ----------------------------------------------------------------------------

def _make_blockdiag(a):
    bd = np.zeros((H, NH), np.float32)
    for h in range(NH):
        bd[h * HD:(h + 1) * HD, h] = a[h]
    return bd


def run_gnn(inputs, n_tiles_per_core=52, trace=False):
    import ml_dtypes
    bf16 = ml_dtypes.bfloat16

    t_start = time.time()
    obs = np.asarray(inputs["obs"], np.float32)
    src = np.asarray(inputs["src"], np.int64)
    dst = np.asarray(inputs["dst"], np.int64)
    N = obs.shape[0]

    NTOT_ = NCORES * n_tiles_per_core * P
    split = min(SPLIT, NTOT_ // 2)
    prep = _host_prepare(src, dst, n_tiles_per_core, split)
    NT = n_tiles_per_core
    TL, TH, NPC, NTOT = prep["TL"], prep["TH"], prep["NPC"], prep["NTOT"]
    perm = prep["perm"]

    al1bd = _make_blockdiag(np.asarray(inputs["al1"], np.float32))
    ar1bd = _make_blockdiag(np.asarray(inputs["ar1"], np.float32))
    al2bd = _make_blockdiag(np.asarray(inputs["al2"], np.float32))
    ar2bd = _make_blockdiag(np.asarray(inputs["ar2"], np.float32))
    Wg1 = np.asarray(inputs["Wg1"], np.float32)
    Wg2 = np.asarray(inputs["Wg2"], np.float32)
    shared = {
        "w1": np.asarray(inputs["W1"], np.float32),
        "b1": np.asarray(inputs["b1"], np.float32).reshape(HID, 1),
        "w2": np.asarray(inputs["W2"], np.float32),
        "b2": np.asarray(inputs["b2"], np.float32).reshape(H, 1),
        "wg1": Wg1.astype(bf16), "wg2": Wg2.astype(bf16),
        "wgal1": (Wg1 @ al1bd).astype(bf16),
        "wgar1": (Wg1 @ ar1bd).astype(bf16),
        "wgal2": (Wg2 @ al2bd).astype(bf16),
        "wgar2": (Wg2 @ ar2bd).astype(bf16),
        "iota": np.tile(np.arange(P, dtype=np.float32)[None, :],
                        (P, 1)).astype(bf16),
        "identf": np.eye(P, dtype=np.float32),
        "iotac": np.arange(P, dtype=np.float32).reshape(P, 1).astype(bf16),
    }

    obs_pad = np.zeros((NTOT, OBS_D), np.float32)
    obs_pad[:N] = obs
    obs_perm = obs_pad[perm]

    in_maps = []
    for c in range(NCORES):
        m = dict(shared)
        m["obst"] = np.ascontiguousarray(obs_perm[c * NPC:(c + 1) * NPC].T)
        m["idxlow"] = prep["idxlow"][c]
        m["idxhigh"] = prep["idxhigh"][c]
        m["dprow"] = prep["dprow"][c]
        m["dposslab"] = prep["dpos"][c]
        in_maps.append(m)

    t_prep = time.time()
    nc = _build_program(NT, TL, TH, split)
    t_build = time.time()

    from concourse.bass_utils import run_bass_kernel_spmd
    if trace:
        _ensure_ntff_hook()
    res = run_bass_kernel_spmd(nc, in_maps, core_ids=list(range(NCORES)),
                               trace=trace)
    t_run = time.time()

    full = np.concatenate([res.results[c]["out"] for c in range(NCORES)],
                          axis=0)
    out = np.empty((N, 3 * H), np.float32)
    keep = perm < N
    out[perm[keep]] = full[keep]

    LAST_INFO.clear()
    LAST_INFO.update(dict(
        exec_time_ns=res.exec_time_ns, TL=TL, TH=TH,
        binload_max=int(prep["binload"].max()),
        t_prep=t_prep - t_start, t_build=t_build - t_prep,
        t_run=t_run - t_build,
        profile_json=getattr(res, "profile_json", None),
    ))
    return out


def kernel(**inputs):
    return run_gnn(inputs, n_tiles_per_core=52,
                   trace=bool(os.environ.get("GNN_TRACE")))
